# revision 1
# baseline (speedup 1.0000x reference)
"""CRF forward (logsumexp over paths) loss kernel for Trainium2, 8 NeuronCores.

Math
----
reference:  fv0 = alpha_0^T + emits[0]                       [B, K]
            fv_t[b,j] = logsumexp_i(fv_{t-1}[b,i] + trans[i,j]) + emit_t[b,j]
            alpha_z = sum_b logsumexp_k( fv_{tau_b}[b,:] )   (tau = one-hot mask step)

We run the recurrence in exp space.  With ETs[i,j] = exp(trans[i,j] - delta)
and e_t[j,b] = exp(emit_t[b,j]) (transposed), the state w_t[j,b] =
exp(fv_t[j,b] - delta*t - C[b]) obeys

    w_t = (ETs^T w_{t-1}) * e_t        (one matmul + one elementwise mul)

C[b] tracks periodic renormalizations (every W=8 steps we divide by a recent
column sum and add its log to C).  The transition weight matrix is augmented
with a 65th column of ones so each matmul also emits colsum(w_{t-1}) in PSUM
row 64; the elementwise multiply covers 65 rows (the transposed-emission tile
has a preset row of ones), so every step's column sum is captured into a
history buffer for free.  The one-hot time mask turns the final
"select alpha at tau_b" into a linear masked sum over that colsum history:

    result[b] = log( sum_s mask[s-1,b] * colsum_{s-1}[b] ) + C_win(s)[b] + delta*tau_b

Sharding: batch B=512 split across 8 cores (64 per core); transitions/alpha_0
replicated; final alpha_z = host sum of the 8 per-core [1,64] row outputs.
"""

import os
import sys

for _p in ("/opt/trn_rl_repo", "/root/.axon_site/_ro/trn_rl_repo"):
    if os.path.isdir(_p) and _p not in sys.path:
        sys.path.insert(0, _p)

from contextlib import ExitStack

import numpy as np

import concourse.bass as bass
import concourse.mybir as mybir
import concourse.tile as tile
from concourse.bass_utils import run_bass_kernel_spmd
from concourse.masks import make_identity

# The walrus build in this container rejects instructions carrying more than
# one sync-wait command ("Too many sync wait commands" in setupSyncWait).
# Tile freely emits multi-wait instructions, so split the extras onto
# preceding same-engine no-ops at commit time (engine queues execute
# in-order, so the semantics are identical).
_ORIG_COMMIT = tile.TileContext._commit_instruction


def _single_wait_commit(self, inst, lazy_reg_writes=True):
    si = getattr(inst, "sync_info", None)
    if (
        si is not None
        and si.on_wait
        and len(si.on_wait) > 1
        and inst.engine != mybir.EngineType.Unassigned
    ):
        waits = list(si.on_wait)
        eng = self.nc.engines[inst.engine]
        for w in waits[:-1]:
            n = eng.nop(nofuse=True)
            n.ins.sync_info = mybir.SyncInfo(on_wait=[w], on_update=[])
        inst.sync_info = mybir.SyncInfo(
            on_wait=[waits[-1]], on_update=list(si.on_update or [])
        )
    _ORIG_COMMIT(self, inst, lazy_reg_writes)


tile.TileContext._commit_instruction = _single_wait_commit

T, B, K = 512, 512, 64
NCORES = 8
BSH = B // NCORES          # 64 batch elements per core
W = 8                      # slots per window (renorm/capture period)
NWINCHAIN = T // W         # 64 windows of chain steps (slots 0..511)
NWIN = NWINCHAIN + 1       # 65: slot 512 (colsum of t=511) lands in window 64
DELTA = 5.0                # static per-step log-space offset folded into ETs
ETRBUF = 24                # transposed-emission ring slots
F32 = mybir.dt.float32
BF16 = mybir.dt.bfloat16
U8 = mybir.dt.uint8
I32 = mybir.dt.int32
MULT = mybir.AluOpType.mult
ADD = mybir.AluOpType.add
AX = mybir.AxisListType.X
AF = mybir.ActivationFunctionType


def _build_crf_nc() -> bass.Bass:
    nc = bass.Bass(trn_type="TRN2", target_bir_lowering=False, debug=False)

    emits_d = nc.dram_tensor("emits", [T, BSH, K], F32, kind="ExternalInput").ap()
    mask_d = nc.dram_tensor("maskb", [T, BSH], U8, kind="ExternalInput").ap()
    trans_d = nc.dram_tensor("transitions", [K, K], F32, kind="ExternalInput").ap()
    alpha0_d = nc.dram_tensor("alpha_0", [K, 1], F32, kind="ExternalInput").ap()
    out_d = nc.dram_tensor("out_row", [1, BSH], F32, kind="ExternalOutput").ap()

    with tile.TileContext(nc) as tc:
        with ExitStack() as ctx:
            _crf_body(ctx, tc, emits_d, mask_d, trans_d, alpha0_d, out_d)
    _split_remaining_multiwaits(nc)
    return nc


def _split_remaining_multiwaits(nc):
    """Split multi-wait instructions added outside the commit path (e.g. the
    end-of-kernel drain/barrier) onto preceding same-engine no-ops."""
    for blk in nc.m.functions[0].blocks:
        il = blk.instructions
        idx = 0
        while idx < len(il):
            inst = il[idx]
            si = inst.sync_info
            if si is not None and si.on_wait and len(si.on_wait) > 1:
                waits = list(si.on_wait)
                for j, w in enumerate(waits[:-1]):
                    n = mybir.InstNoOp(
                        name=f"I-swx-{inst.name}-{j}", ins=[], outs=[]
                    )
                    n.engine = inst.engine
                    n.sync_info = mybir.SyncInfo(on_wait=[w], on_update=[])
                    nc.register_instruction(n, overwrite=True)
                    il.insert(idx, n)
                    idx += 1
                inst.sync_info = mybir.SyncInfo(
                    on_wait=[waits[-1]], on_update=list(si.on_update or [])
                )
            idx += 1


def _crf_body(ctx, tc, emits_d, mask_d, trans_d, alpha0_d, out_d):
    nc = tc.nc

    # ---- long-lived SBUF state ----
    ets = nc.alloc_sbuf_tensor("ets", [K, K + 1], BF16).ap()        # exp(trans-d)|1
    expal = nc.alloc_sbuf_tensor("expal", [K, 1], F32).ap()        # exp(alpha_0)
    # chain state ring: 2 window buffers x W slots x BSH cols, 65 rows
    # (row 64 of slot s = colsum of w_{s-1})
    w_all = nc.alloc_sbuf_tensor("w_all", [K + 1, 2 * W * BSH], BF16).ap()
    wrn = nc.alloc_sbuf_tensor("wrn", [K, 2 * BSH], BF16).ap()      # renormed state
    c_rows = nc.alloc_sbuf_tensor("c_rows", [1, 2 * BSH], F32).ap()  # C ping-pong
    en_ring = nc.alloc_sbuf_tensor("en_ring", [BSH, 2 * W * 2 * K], BF16).ap()
    ident = nc.alloc_sbuf_tensor("ident", [BSH, BSH], BF16).ap()
    etr_sb = nc.alloc_sbuf_tensor("etr_sb", [K + 1, 2 * W * BSH], BF16).ap()
    csum = nc.alloc_sbuf_tensor("csum", [NWIN, W * BSH], BF16).ap()  # colsum history
    c_hist = nc.alloc_sbuf_tensor("c_hist", [NWIN, BSH], F32).ap()  # log-norm per win
    maskw = nc.alloc_sbuf_tensor("maskw", [NWIN, W * BSH], F32).ap()
    mk_u8 = nc.alloc_sbuf_tensor("mk_u8", [NWIN, W * BSH], U8).ap()
    iota_i = nc.alloc_sbuf_tensor("iota_i", [NWIN, W * BSH], I32).ap()
    iotaw = nc.alloc_sbuf_tensor("iotaw", [NWIN, W * BSH], F32).ap()
    ones_c = nc.alloc_sbuf_tensor("ones_c", [NWIN, 1], F32).ap()   # partition-reduce
    ones_r = nc.alloc_sbuf_tensor("ones_r", [1, K], F32).ap()      # row broadcast
    cst = nc.alloc_sbuf_tensor("cst", [K, 2], F32).ap()            # bias constants

    # ---- pools ----
    em_pool = ctx.enter_context(tc.tile_pool(name="em", bufs=3))
    etp_pool = ctx.enter_context(tc.tile_pool(name="etp", bufs=3, space="PSUM"))
    ps_pool = ctx.enter_context(tc.tile_pool(name="ps", bufs=3, space="PSUM"))
    psb_pool = ctx.enter_context(tc.tile_pool(name="psb", bufs=2, space="PSUM"))
    row_pool = ctx.enter_context(tc.tile_pool(name="rows", bufs=6))
    fin_pool = ctx.enter_context(tc.tile_pool(name="fin", bufs=1))

    # ---- one-time setup ----
    nc.vector.memset(w_all[K : K + 1, 0:BSH], 0.0)  # slot 0 has no colsum
    # emission staging: per step a [BSH, 128] block, col 64 = 1.0 (becomes the
    # ones row of the transposed tile -> colsum row of the state)
    nc.gpsimd.memset(en_ring[:, :], 0.0)
    nc.vector.memset(
        en_ring.rearrange("b (s c) -> b s c", c=2 * K)[:, :, K : K + 1], 1.0
    )
    nc.gpsimd.memset(csum[:, :], 0.0)
    nc.gpsimd.memset(c_hist[:, :], 0.0)
    nc.gpsimd.memset(c_rows[:, :], 0.0)
    nc.gpsimd.memset(mk_u8[:, :], 0)
    nc.gpsimd.memset(ones_c[:, :], 1.0)
    nc.gpsimd.memset(ones_r[:, :], 1.0)
    nc.gpsimd.memset(cst[:, 0:1], 0.0)
    nc.gpsimd.memset(cst[:, 1:2], -DELTA)
    make_identity(nc, ident)

    tr_t = fin_pool.tile([K, K], F32)
    nc.sync.dma_start(tr_t[:], trans_d)
    nc.scalar.activation(ets[:, 0:K], tr_t[:], AF.Exp, bias=cst[0:K, 1:2])
    nc.vector.memset(ets[:, K : K + 1], 1.0)

    a0_t = fin_pool.tile([K, 1], F32)
    nc.sync.dma_start(a0_t[:], alpha0_d)
    nc.scalar.activation(expal, a0_t[:], AF.Exp, bias=cst[0:K, 0:1])

    # mask (one-hot over t, per b) -> slot layout: slot s <-> t = s-1.
    # maskw[win, tw*BSH + b] = mask[win*W + tw - 1, b]
    nc.sync.dma_start(
        mk_u8[0:1, BSH : W * BSH],
        mask_d[0 : W - 1].rearrange("(o t) b -> o (t b)", o=1),
    )
    nc.sync.dma_start(
        mk_u8[1:NWINCHAIN, :],
        mask_d[W - 1 : T - 1].rearrange("(w t) b -> w t b", t=W),
    )
    nc.sync.dma_start(mk_u8[NWINCHAIN : NWIN, 0:BSH], mask_d[T - 1 : T])
    nc.vector.tensor_copy(maskw[:, :], mk_u8[:, :])
    # t value at each slot position (t = win*W + tw - 1)
    nc.gpsimd.iota(iota_i[:, :], pattern=[[1, W], [0, BSH]], base=-1,
                   channel_multiplier=W)
    nc.vector.tensor_copy(iotaw[:, :], iota_i[:, :])

    def w_off(t):  # column offset of chain slot t in w_all
        return ((t // W) % 2) * (W * BSH) + (t % W) * BSH

    def en_blk(t):
        par = (t // W) % 2
        v = en_ring.rearrange("b (s c) -> b s c", c=2 * K)
        return v[:, par * W + (t % W), :]

    def etr_sl(t):
        o = ((t // W) % 2) * (W * BSH) + (t % W) * BSH
        return etr_sb[:, o : o + BSH]

    # ---- emission streaming: load chunk of W steps, exp -> bf16, transpose ----
    def load_chunk(t0):
        em = em_pool.tile([BSH, W * K], F32)
        nc.sync.dma_start(
            em[:].rearrange("b (t k) -> b t k", t=W),
            emits_d[t0 : t0 + W].rearrange("t b k -> b t k"),
        )
        par = (t0 // W) % 2
        blk = en_ring.rearrange("b (s c) -> b s c", c=2 * K)[
            :, par * W : (par + 1) * W, :
        ]
        nc.scalar.activation(
            blk[:, :, 0:K],
            em[:].rearrange("b (t k) -> b t k", t=W),
            AF.Exp,
            bias=cst[0:BSH, 0:1],
        )

    # ---- chain ----
    # Per window of W steps: 8 PE transposes land in ONE PSUM tile (columns),
    # then one ACT copy moves the whole window to SBUF.  PE transposes for
    # window w+1 are interleaved one-per-chain-step so they hide in the PE
    # idle time of the serial chain.
    load_chunk(0)
    load_chunk(W)
    etp_cur = etp_pool.tile([K + 1, W * BSH], BF16, tag="etp")
    for tw in range(W):
        nc.tensor.transpose(
            etp_cur[:, tw * BSH : (tw + 1) * BSH], en_blk(tw)[:, 0 : K + 1], ident
        )
    nc.scalar.copy(etr_sb[:, 0 : W * BSH], etp_cur[:])
    # t = 0: w_0 = exp(alpha_0) * e_0
    nc.vector.tensor_scalar(
        w_all[0:K, 0:BSH], etr_sl(0)[0:K, :], expal, None, op0=MULT
    )
    etp_cur = etp_pool.tile([K + 1, W * BSH], BF16, tag="etp")  # window 1
    nc.tensor.transpose(etp_cur[:, 0:BSH], en_blk(W)[:, 0 : K + 1], ident)

    HB = BSH // 2  # sub-chain half width
    bc_cur = None
    for t in range(1, T):
        if t % W == 0 and t + 2 * W <= T:
            load_chunk(t + W)
        if t % W == 0 and t + W < T:
            etp_cur = etp_pool.tile([K + 1, W * BSH], BF16, tag="etp")
        s = t + W  # lookahead transpose for slot s
        if s < T:
            nc.tensor.transpose(
                etp_cur[:, (s % W) * BSH : (s % W + 1) * BSH],
                en_blk(s)[:, 0 : K + 1],
                ident,
            )
        if t % W == 4:
            # stale renorm prep (off the critical path): reciprocal +
            # broadcast of colsum_{t-2} (slot t-1's row 64); the scale is
            # applied at the window boundary and ln(s) booked into C.
            win = t // W
            woff = (win % 2) * (W * BSH)
            s_row = w_all[K : K + 1, woff + 3 * BSH : woff + 4 * BSH]
            ln_s = row_pool.tile([1, BSH], F32, tag="lns")
            nc.scalar.activation(ln_s[:], s_row, AF.Ln, bias=cst[0:1, 0:1])
            # 1/s as exp(-ln s) on the Scalar engine: keeps the reciprocal
            # off the busy DVE, and the booked ln_s matches the applied
            # factor by construction.
            rc = row_pool.tile([1, BSH], F32, tag="recip")
            nc.scalar.activation(rc[:], ln_s[:], AF.Exp, scale=-1.0,
                                 bias=cst[0:1, 0:1])
            bc_cur = psb_pool.tile([K, BSH], F32, tag="bc")
            nc.tensor.matmul(bc_cur[:], ones_r, rc[:], start=True, stop=True)
            pw, pr = ((win + 1) % 2) * BSH, (win % 2) * BSH
            nc.gpsimd.tensor_tensor(
                c_rows[:, pw : pw + BSH], c_rows[:, pr : pr + BSH], ln_s[:], op=ADD
            )
            nc.gpsimd.dma_start(c_hist[win + 1 : win + 2, :], c_rows[:, pw : pw + BSH])
        # rhs = previous state (renormed copy at window starts)
        if t % W == 0:
            wp = wrn[:, ((t // W - 1) % 2) * BSH :]
        else:
            o = w_off(t - 1)
            wp = w_all[0:K, o : o + BSH]
        o = w_off(t)
        # two independent 32-wide sub-chains hide each other's latency
        ps_a = ps_pool.tile([K + 1, HB], F32, tag="ps")
        nc.tensor.matmul(ps_a[:], ets[:, :], wp[0:K, 0:HB], start=True, stop=True)
        ps_b = ps_pool.tile([K + 1, HB], F32, tag="ps")
        nc.tensor.matmul(ps_b[:], ets[:, :], wp[0:K, HB:BSH], start=True, stop=True)
        nc.vector.tensor_tensor(
            w_all[0 : K + 1, o : o + HB], ps_a[:], etr_sl(t)[:, 0:HB], op=MULT
        )
        nc.vector.tensor_tensor(
            w_all[0 : K + 1, o + HB : o + BSH], ps_b[:], etr_sl(t)[:, HB:BSH],
            op=MULT,
        )
        if t % W == W - 1 and s < T:
            # window (t+W)//W fully transposed -> batch copy to SBUF
            wn = s // W
            nc.scalar.copy(
                etr_sb[:, (wn % 2) * W * BSH : ((wn % 2) + 1) * W * BSH],
                etp_cur[:],
            )

        if t % W == W - 1:
            win = t // W
            woff = (win % 2) * (W * BSH)
            # capture the window's colsum row into the history (DMA: compute
            # engines cannot write at arbitrary start partitions)
            nc.gpsimd.dma_start(
                csum[win : win + 1, :], w_all[K : K + 1, woff : woff + W * BSH]
            )
            # apply the pre-computed stale renorm scale
            nc.vector.tensor_tensor(
                wrn[:, (win % 2) * BSH :][:, 0:BSH],
                w_all[0:K, woff + (W - 1) * BSH : woff + W * BSH],
                bc_cur[:],
                op=MULT,
            )

    # slot 512: one extra matmul for colsum of w_{511}
    ps = ps_pool.tile([K + 1, BSH], F32, tag="ps")
    nc.tensor.matmul(
        ps[:], ets[:, :], wrn[:, (NWINCHAIN - 1) % 2 * BSH :][:, 0:BSH],
        start=True, stop=True,
    )
    nc.scalar.copy(csum[NWINCHAIN : NWIN, 0:BSH], ps[K : K + 1, :])

    # ---- final combine ----
    prodz = fin_pool.tile([NWIN, W * BSH], F32, tag="prodz")
    nc.vector.tensor_tensor(prodz[:], csum[:, :], maskw[:, :], op=MULT)
    redz = fin_pool.tile([NWIN, BSH], F32, tag="redz")
    nc.vector.tensor_reduce(
        redz[:], prodz[:].rearrange("p (t b) -> p b t", t=W), axis=AX, op=ADD
    )
    mwin = fin_pool.tile([NWIN, BSH], F32, tag="mwin")
    nc.vector.tensor_reduce(
        mwin[:], maskw[:, :].rearrange("p (t b) -> p b t", t=W), axis=AX, op=ADD
    )
    xc = fin_pool.tile([NWIN, BSH], F32, tag="xc")
    nc.vector.tensor_tensor(xc[:], mwin[:], c_hist[:, :], op=MULT)
    prodt = fin_pool.tile([NWIN, W * BSH], F32, tag="prodt")
    nc.vector.tensor_tensor(prodt[:], maskw[:, :], iotaw[:, :], op=MULT)
    redt = fin_pool.tile([NWIN, BSH], F32, tag="redt")
    nc.vector.tensor_reduce(
        redt[:], prodt[:].rearrange("p (t b) -> p b t", t=W), axis=AX, op=ADD
    )
    # xc += DELTA * redt
    nc.vector.scalar_tensor_tensor(xc[:], redt[:], DELTA, xc[:], op0=MULT, op1=ADD)

    accz = ps_pool.tile([1, BSH], F32, tag="ps")
    nc.tensor.matmul(accz[:], ones_c, redz[:], start=True, stop=True)
    accc = ps_pool.tile([1, BSH], F32, tag="ps")
    nc.tensor.matmul(accc[:], ones_c, xc[:], start=True, stop=True)
    lnz = row_pool.tile([1, BSH], F32, tag="lnz")
    nc.scalar.activation(lnz[:], accz[:], AF.Ln, bias=cst[0:1, 0:1])
    res = row_pool.tile([1, BSH], F32, tag="res")
    nc.vector.tensor_tensor(res[:], lnz[:], accc[:], op=ADD)
    nc.sync.dma_start(out_d, res[:])


_NC_CACHE = None


def _get_nc():
    global _NC_CACHE
    if _NC_CACHE is None:
        _NC_CACHE = _build_crf_nc()
    return _NC_CACHE


def _make_in_maps(np_inputs):
    emits = np.asarray(np_inputs["emits"], dtype=np.float32)
    mask_u8 = np.asarray(np_inputs["mask"]).astype(np.uint8)
    transitions = np.asarray(np_inputs["transitions"], dtype=np.float32)
    alpha_0 = np.asarray(np_inputs["alpha_0"], dtype=np.float32)
    in_maps = []
    for c in range(NCORES):
        sl = slice(c * BSH, (c + 1) * BSH)
        in_maps.append(
            {
                "emits": np.ascontiguousarray(emits[:, sl, :]),
                "maskb": np.ascontiguousarray(mask_u8[:, sl]),
                "transitions": transitions,
                "alpha_0": alpha_0,
            }
        )
    return in_maps


def kernel(emits, mask, transitions, alpha_0):
    nc = _get_nc()
    in_maps = _make_in_maps(
        {"emits": emits, "mask": mask, "transitions": transitions,
         "alpha_0": alpha_0}
    )
    res = run_bass_kernel_spmd(nc, in_maps, core_ids=list(range(NCORES)))
    total = np.float64(0.0)
    for r in res.results:
        total += np.asarray(r["out_row"], dtype=np.float64).sum()
    return np.float32(total)



# revision 14
# speedup vs baseline: 1.7156x; 1.7156x over previous
"""CRF forward (logsumexp over paths) loss kernel for Trainium2, 8 NeuronCores.

Time-parallel chunked algorithm
-------------------------------
The linear-space recurrence  w_t = (ETs^T w_{t-1}) * e_t  (ETs = exp(trans-D),
e_t = exp(emit_t), state [K, B] per core) is a product of positive matrices,
so it forgets its initial condition at the Birkhoff contraction rate —
measured here at ~3 decades per 4 steps.  That lets the T=512 serial chain be
cut into S=32 time chunks run CONCURRENTLY: each chunk starts from the
uniform state w := e_{t0} a few steps (m=4) before its real range and is
correct in *direction* by the time the range starts; its unknown per-batch
log-magnitude offset delta_c is recovered afterwards by matching log-colsums
with the previous chunk at the shared boundary step (a tiny scalar cumsum).

Per core (64-batch shard), chunks run in 4 groups of 8 batched into the free
axis: one [65, 512] matmul per group-step whose 65th weight column of ones
emits the colsum row Z for free; the elementwise emission multiply covers 65
rows (the staged emission tile has a preset ones row), so Z lands in the
SBUF state tile and streams to a Z history via Pool-issued DMAs (25ns issue).
Emissions are host-transposed to [T, K, B] so they DMA straight into [k, b]
layout — no on-device transposes — and exp'd on the Scalar engine.  The
multiply is split DVE/Pool by column range to use both engines.
Final combine: per-chunk Z histories are PE-transposed to [b, slot], matched
into delta_c (log-ratio cumsum), and the one-hot time mask (host-preprocessed
into one-hot (chunk,slot) + chunk indicators) selects
ln Z(tau_b) + delta_c(b) + D*tau_b; a ones-matmul reduces the batch on core.

Sharding: batch 512 = 8 cores x 64, transitions/alpha_0 replicated; host sums
the 8 per-core scalars.
"""

import os
import sys

for _p in ("/opt/trn_rl_repo", "/root/.axon_site/_ro/trn_rl_repo"):
    if os.path.isdir(_p) and _p not in sys.path:
        sys.path.insert(0, _p)

from contextlib import ExitStack

import numpy as np

import concourse.bass as bass
import concourse.mybir as mybir
import concourse.tile as tile
from concourse.bass_utils import run_bass_kernel_spmd
from concourse.masks import make_identity

# Walrus in this container rejects instructions with >1 sync-wait; split the
# extras onto preceding same-engine no-ops (queues are in-order, so identical
# semantics).
_ORIG_COMMIT = tile.TileContext._commit_instruction


def _single_wait_commit(self, inst, lazy_reg_writes=True):
    si = getattr(inst, "sync_info", None)
    if (
        si is not None
        and si.on_wait
        and len(si.on_wait) > 1
        and inst.engine != mybir.EngineType.Unassigned
    ):
        waits = list(si.on_wait)
        eng = self.nc.engines[inst.engine]
        for w in waits[:-1]:
            n = eng.nop(nofuse=True)
            n.ins.sync_info = mybir.SyncInfo(on_wait=[w], on_update=[])
        inst.sync_info = mybir.SyncInfo(
            on_wait=[waits[-1]], on_update=list(si.on_update or [])
        )
    _ORIG_COMMIT(self, inst, lazy_reg_writes)


tile.TileContext._commit_instruction = _single_wait_commit

T, B, K = 512, 512, 64
NCORES = 8
BSH = B // NCORES      # 64 batch per core
P = 16                 # real steps per chunk
M = 4                  # burn-in steps
S = T // P             # 32 chunks
LL = P + M             # 20 chain steps per chunk (zbuf rows 1..LL)
NR = LL + 1            # 21 zbuf rows (row 0 unused, kept 1.0)
NR2 = NR + 1           # 22: zT column pitch (4-byte aligned for bf16 PSUM)
NG = 4                 # groups
G = S // NG            # 8 chunks per group
GC = G * BSH           # 512 columns per group
W = 4                  # emission window (steps per DMA/exp block)
NW = LL // W           # 5 windows
DSPLIT = 320           # mult columns 0..DSPLIT-1 on DVE, rest on Pool
DELTA = 4.0            # per-step log-space offset folded into ETs
F32 = mybir.dt.float32
BF16 = mybir.dt.bfloat16
MULT = mybir.AluOpType.mult
ADD = mybir.AluOpType.add
SUB = mybir.AluOpType.subtract
AX = mybir.AxisListType.X
AF = mybir.ActivationFunctionType


def _t_start(c):
    return 0 if c == 0 else c * P - M


def _build_crf_nc() -> bass.Bass:
    nc = bass.Bass(trn_type="TRN2", target_bir_lowering=False, debug=False)

    # emits host-transposed to [T, K, BSH] so DMA lands in [k, b] layout
    emt_d = nc.dram_tensor("emits_t", [T, K, BSH], F32, kind="ExternalInput").ap()
    trans_d = nc.dram_tensor("transitions", [K, K], F32, kind="ExternalInput").ap()
    alpha0_d = nc.dram_tensor("alpha_0", [K, 1], F32, kind="ExternalInput").ap()
    ohz_d = nc.dram_tensor("onehot_z", [BSH, S * NR2], F32, kind="ExternalInput").ap()
    ohc_d = nc.dram_tensor("onehot_c", [BSH, S], F32, kind="ExternalInput").ap()
    taud_d = nc.dram_tensor("tau_delta", [BSH, 1], F32, kind="ExternalInput").ap()
    out_d = nc.dram_tensor("out_sum", [1, 1], F32, kind="ExternalOutput").ap()

    with tile.TileContext(nc) as tc:
        with ExitStack() as ctx:
            _crf_body(ctx, tc, emt_d, trans_d, alpha0_d, ohz_d, ohc_d, taud_d,
                      out_d)
    _split_remaining_multiwaits(nc)
    return nc


def _split_remaining_multiwaits(nc):
    for blk in nc.m.functions[0].blocks:
        il = blk.instructions
        idx = 0
        while idx < len(il):
            inst = il[idx]
            si = inst.sync_info
            if si is not None and si.on_wait and len(si.on_wait) > 1:
                waits = list(si.on_wait)
                for j, w in enumerate(waits[:-1]):
                    n = mybir.InstNoOp(
                        name=f"I-swx-{inst.name}-{j}", ins=[], outs=[]
                    )
                    n.engine = inst.engine
                    n.sync_info = mybir.SyncInfo(on_wait=[w], on_update=[])
                    nc.register_instruction(n, overwrite=True)
                    il.insert(idx, n)
                    idx += 1
                inst.sync_info = mybir.SyncInfo(
                    on_wait=[waits[-1]], on_update=list(si.on_update or [])
                )
            idx += 1


def _crf_body(ctx, tc, emt_d, trans_d, alpha0_d, ohz_d, ohc_d, taud_d, out_d):
    nc = tc.nc

    # ---- long-lived SBUF ----
    ets = nc.alloc_sbuf_tensor("ets", [K, K + 1], BF16).ap()
    expal = nc.alloc_sbuf_tensor("expal", [K + 1, 1], F32).ap()
    identf = nc.alloc_sbuf_tensor("identf", [NR + 1, NR + 1], BF16).ap()
    ones_b = nc.alloc_sbuf_tensor("ones_b", [BSH, 1], F32).ap()
    cst = nc.alloc_sbuf_tensor("cst", [128, 2], F32).ap()  # col0=0, col1=-DELTA
    zbuf = [
        nc.alloc_sbuf_tensor(f"zbuf{g}", [NR, GC], BF16).ap() for g in range(NG)
    ]
    # emission staging: per group 2 persistent buffers [65, W*GC] bf16 with
    # row 64 = 1.0 (preset once) so the 65-row multiply passes Z through.
    eexp = [
        [nc.alloc_sbuf_tensor(f"eexp{g}_{i}", [K + 1, W * GC], BF16).ap()
         for i in range(2)]
        for g in range(NG)
    ]

    # ---- one-time setup ----
    nc.gpsimd.memset(cst[:, 0:1], 0.0)
    nc.gpsimd.memset(cst[:, 1:2], -DELTA)
    nc.gpsimd.memset(ones_b[:, :], 1.0)
    for g in range(NG):
        nc.vector.memset(zbuf[g][:, :], 1.0)  # row 0 stays 1.0 -> ln = 0
        for i in range(2):
            nc.vector.memset(eexp[g][i][K : K + 1, :], 1.0)
    make_identity(nc, identf)

    fin_pool = ctx.enter_context(tc.tile_pool(name="fin", bufs=1))

    tr_t = fin_pool.tile([K, K], F32, tag="trt")
    nc.sync.dma_start(tr_t[:], trans_d)
    nc.scalar.activation(ets[:, 0:K], tr_t[:], AF.Exp, bias=cst[0:K, 1:2])
    nc.vector.memset(ets[:, K : K + 1], 1.0)

    a0_t = fin_pool.tile([K, 1], F32, tag="a0t")
    nc.sync.dma_start(a0_t[:], alpha0_d)
    nc.scalar.activation(expal[0:K], a0_t[:], AF.Exp, bias=cst[0:K, 0:1])
    nc.vector.memset(expal[K : K + 1], 1.0)

    ohz = fin_pool.tile([BSH, S * NR2], F32, tag="ohz")
    nc.sync.dma_start(ohz[:], ohz_d)
    ohc = fin_pool.tile([BSH, S], F32, tag="ohc")
    nc.sync.dma_start(ohc[:], ohc_d)
    taud = fin_pool.tile([BSH, 1], F32, tag="taud")
    nc.sync.dma_start(taud[:], taud_d)

    with ExitStack() as chain_ctx:
        raw_pool = chain_ctx.enter_context(tc.tile_pool(name="raw", bufs=2))
        w_pool = chain_ctx.enter_context(tc.tile_pool(name="wst", bufs=3))
        u_psum = chain_ctx.enter_context(
            tc.tile_pool(name="upsum", bufs=2, space="PSUM")
        )

        # window DMA + exp: raw [64, W*GC] f32, chunk-major: col =
        # chunk*(W*64) + sw*64 + b  (keeps each DMA a simple 2-D pattern)
        def load_window(g, wv):
            rt = raw_pool.tile([K, W * GC], F32, tag=f"raw{g}")
            for ci in range(G):
                c = g * G + ci
                t0 = _t_start(c) + wv * W
                nc.gpsimd.dma_start(
                    rt[:, ci * W * BSH : (ci + 1) * W * BSH].rearrange(
                        "k (t b) -> k t b", t=W
                    ),
                    emt_d[t0 : t0 + W].rearrange("t k b -> k t b"),
                )
            dst = eexp[g][wv % 2]
            nc.scalar.activation(dst[0:K, :], rt[:], AF.Exp, bias=cst[0:K, 0:1])

        for g in range(NG):
            load_window(g, 0)
        for g in range(NG):
            load_window(g, 1)

        # init states: w0 = e_{t0} (chunks >=1), chunk 0: expal * e_0
        wcur = [None] * NG
        for g in range(NG):
            wt = w_pool.tile([K + 1, GC], BF16, tag=f"w{g}")
            sv = eexp[g][0][:].rearrange("k (c t b) -> k c t b", c=G, t=W)
            wv_ = wt[:].rearrange("k (c b) -> k c b", c=G)
            if g == 0:
                nc.vector.tensor_scalar(
                    wv_[:, 0, :], sv[:, 0, 0, :], expal, None, op0=MULT
                )
                nc.vector.tensor_copy(wv_[:, 1:G, :], sv[:, 1:G, 0, :])
            else:
                nc.vector.tensor_copy(wv_[:, :, :], sv[:, :, 0, :])
            wcur[g] = wt

        # ---- chain: steps 1..LL ----
        for s in range(1, LL + 1):
            if s % W == 0 and s // W + 1 < NW:
                for g in range(NG):
                    load_window(g, s // W + 1)
            se = min(s, LL - 1)        # step LL reuses step LL-1's emission
            wv, sw = se // W, se % W
            for g in range(NG):
                u = u_psum.tile([K + 1, GC], F32, tag=f"u{g}")
                nc.tensor.matmul(
                    u[:], ets[:, :], wcur[g][0:K, :], start=True, stop=True
                )
                sv = eexp[g][wv % 2][:].rearrange(
                    "k (c t b) -> k c t b", c=G, t=W
                )
                wt = w_pool.tile([K + 1, GC], BF16, tag=f"w{g}")
                nc.vector.tensor_tensor(
                    wt[:].rearrange("k (c b) -> k c b", c=G),
                    u[:].rearrange("k (c b) -> k c b", c=G),
                    sv[:, :, sw, :],
                    op=MULT,
                )
                # Z(s-1) = row 64 of u (colsum via ets ones column), passed
                # through the multiply by the emission tile's ones row.
                nc.gpsimd.dma_start(
                    zbuf[g][s : s + 1, :], wt[K : K + 1, :]
                )
                wcur[g] = wt

    # ---- final combine ----
    fpsum = ctx.enter_context(tc.tile_pool(name="fpsum", bufs=2, space="PSUM"))
    zT = fin_pool.tile([BSH, S * NR2], F32, tag="zT")
    # pad columns would otherwise hold junk; preset whole tile Ln-safe
    nc.vector.memset(zT[:, :], 1.0)
    for h in range(2):
        zt = fpsum.tile([BSH, (S // 2) * NR2], BF16, tag="zt")
        for ci in range(S // 2):
            c = h * (S // 2) + ci
            g, gi = c // G, c % G
            nc.tensor.transpose(
                zt[:, ci * NR2 : ci * NR2 + NR],
                zbuf[g][:, gi * BSH : (gi + 1) * BSH],
                identf[0:NR, 0:NR],
            )
        nc.vector.tensor_copy(
            zT[:, h * (S // 2) * NR2 : (h + 1) * (S // 2) * NR2].rearrange(
                "b (c r) -> b c r", r=NR2
            )[:, :, 0:NR],
            zt[:].rearrange("b (c r) -> b c r", r=NR2)[:, :, 0:NR],
        )
    # patch: chunk0's matching column (row LL) := its row P (t = P-1)
    nc.vector.tensor_copy(zT[:, LL : LL + 1], zT[:, P : P + 1])
    lnz = fin_pool.tile([BSH, S * NR2], F32, tag="lnz")
    nc.scalar.activation(lnz[:], zT[:], AF.Ln, bias=cst[0:BSH, 0:1])

    # delta stitching: inc[:, i] = lnz[:, NR*(i-1) + LL] - lnz[:, NR*i + M]
    lv = lnz[:].rearrange("b (c r) -> b c r", r=NR2)
    inc = fin_pool.tile([BSH, S], F32, tag="inc")
    nc.vector.memset(inc[:, 0:1], 0.0)
    nc.vector.tensor_tensor(
        inc[:, 1:S], lv[:, 0 : S - 1, LL], lv[:, 1:S, M], op=SUB
    )
    # cumulative sum over chunks (Hillis-Steele, ping-pong)
    cs_a = inc
    for k in (1, 2, 4, 8, 16):
        cs_b = fin_pool.tile([BSH, S], F32, tag=f"cs{k}")
        nc.vector.tensor_copy(cs_b[:, 0:k], cs_a[:, 0:k])
        nc.vector.tensor_tensor(
            cs_b[:, k:S], cs_a[:, k:S], cs_a[:, 0 : S - k], op=ADD
        )
        cs_a = cs_b

    scr1 = fin_pool.tile([BSH, S * NR2], F32, tag="scr1")
    zsel = fin_pool.tile([BSH, 1], F32, tag="zsel")
    nc.vector.tensor_tensor(scr1[:], lnz[:], ohz[:], op=MULT)
    nc.vector.tensor_reduce(zsel[:], scr1[:], axis=AX, op=ADD)
    scr2 = fin_pool.tile([BSH, S], F32, tag="scr2")
    dsel = fin_pool.tile([BSH, 1], F32, tag="dsel")
    nc.vector.tensor_tensor(scr2[:], cs_a[:], ohc[:], op=MULT)
    nc.vector.tensor_reduce(dsel[:], scr2[:], axis=AX, op=ADD)
    res = fin_pool.tile([BSH, 1], F32, tag="res")
    nc.vector.tensor_tensor(res[:], zsel[:], dsel[:], op=ADD)
    nc.vector.tensor_tensor(res[:], res[:], taud[:], op=ADD)
    acc = fpsum.tile([1, 1], F32, tag="acc")
    nc.tensor.matmul(acc[:], res[:], ones_b[:], start=True, stop=True)
    osb = fin_pool.tile([1, 1], F32, tag="osb")
    nc.scalar.copy(osb[:], acc[:])
    nc.sync.dma_start(out_d, osb[:])


_NC_CACHE = None


def _get_nc():
    global _NC_CACHE
    if _NC_CACHE is None:
        _NC_CACHE = _build_crf_nc()
    return _NC_CACHE


def _make_in_maps(np_inputs):
    emits = np.asarray(np_inputs["emits"], dtype=np.float32)
    mask = np.asarray(np_inputs["mask"])
    transitions = np.asarray(np_inputs["transitions"], dtype=np.float32)
    alpha_0 = np.asarray(np_inputs["alpha_0"], dtype=np.float32)
    emits_t = np.ascontiguousarray(emits.transpose(0, 2, 1))  # [T, K, B]
    tau = mask.argmax(0).astype(np.int64)  # [B]
    chunk = tau // P
    row = np.where(chunk == 0, tau + 1, tau % P + M + 1)
    in_maps = []
    for cix in range(NCORES):
        sl = slice(cix * BSH, (cix + 1) * BSH)
        tau_s, c_s, r_s = tau[sl], chunk[sl], row[sl]
        ohz = np.zeros((BSH, S * NR2), dtype=np.float32)
        ohz[np.arange(BSH), c_s * NR2 + r_s] = 1.0
        ohc = np.zeros((BSH, S), dtype=np.float32)
        ohc[np.arange(BSH), c_s] = 1.0
        taud = (DELTA * tau_s).astype(np.float32).reshape(BSH, 1)
        in_maps.append(
            {
                "emits_t": np.ascontiguousarray(emits_t[:, :, sl]),
                "transitions": transitions,
                "alpha_0": alpha_0,
                "onehot_z": ohz,
                "onehot_c": ohc,
                "tau_delta": taud,
            }
        )
    return in_maps


def kernel(emits, mask, transitions, alpha_0):
    nc = _get_nc()
    in_maps = _make_in_maps(
        {"emits": emits, "mask": mask, "transitions": transitions,
         "alpha_0": alpha_0}
    )
    res = run_bass_kernel_spmd(nc, in_maps, core_ids=list(range(NCORES)))
    total = np.float64(0.0)
    for r in res.results:
        total += np.asarray(r["out_sum"], dtype=np.float64).sum()
    return np.float32(total)


# revision 15
# speedup vs baseline: 3.0521x; 1.7790x over previous
"""CRF forward (logsumexp over paths) loss kernel for Trainium2, 8 NeuronCores.

Time-parallel chunked algorithm
-------------------------------
The linear-space recurrence  w_t = (ETs^T w_{t-1}) * e_t  (ETs = exp(trans-D),
e_t = exp(emit_t), state [K, B] per core) is a product of positive matrices,
so it forgets its initial condition at the Birkhoff contraction rate —
measured here at ~3 decades per 4 steps.  That lets the T=512 serial chain be
cut into S=32 time chunks run CONCURRENTLY: each chunk starts from the
uniform state w := e_{t0} a few steps (m=4) before its real range and is
correct in *direction* by the time the range starts; its unknown per-batch
log-magnitude offset delta_c is recovered afterwards by matching log-colsums
with the previous chunk at the shared boundary step (a tiny scalar cumsum).

Per core (64-batch shard), chunks run in 4 groups of 8 batched into the free
axis: one [65, 512] matmul per group-step whose 65th weight column of ones
emits the colsum row Z for free; the elementwise emission multiply covers 65
rows (the staged emission tile has a preset ones row), so Z lands in the
SBUF state tile and streams to a Z history via Pool-issued DMAs (25ns issue).
Emissions are host-transposed to [T, K, B] so they DMA straight into [k, b]
layout — no on-device transposes — and exp'd on the Scalar engine.  The
multiply is split DVE/Pool by column range to use both engines.
Final combine: per-chunk Z histories are PE-transposed to [b, slot], matched
into delta_c (log-ratio cumsum), and the one-hot time mask (host-preprocessed
into one-hot (chunk,slot) + chunk indicators) selects
ln Z(tau_b) + delta_c(b) + D*tau_b; a ones-matmul reduces the batch on core.

Sharding: batch 512 = 8 cores x 64, transitions/alpha_0 replicated; host sums
the 8 per-core scalars.
"""

import os
import sys

for _p in ("/opt/trn_rl_repo", "/root/.axon_site/_ro/trn_rl_repo"):
    if os.path.isdir(_p) and _p not in sys.path:
        sys.path.insert(0, _p)

from contextlib import ExitStack

import numpy as np

import concourse.bass as bass
import concourse.mybir as mybir
import concourse.tile as tile
from concourse.bass_utils import run_bass_kernel_spmd
from concourse.masks import make_identity

# Walrus in this container rejects instructions with >1 sync-wait; split the
# extras onto preceding same-engine no-ops (queues are in-order, so identical
# semantics).
_ORIG_COMMIT = tile.TileContext._commit_instruction


def _single_wait_commit(self, inst, lazy_reg_writes=True):
    si = getattr(inst, "sync_info", None)
    if (
        si is not None
        and si.on_wait
        and len(si.on_wait) > 1
        and inst.engine != mybir.EngineType.Unassigned
    ):
        waits = list(si.on_wait)
        eng = self.nc.engines[inst.engine]
        for w in waits[:-1]:
            n = eng.nop(nofuse=True)
            n.ins.sync_info = mybir.SyncInfo(on_wait=[w], on_update=[])
        inst.sync_info = mybir.SyncInfo(
            on_wait=[waits[-1]], on_update=list(si.on_update or [])
        )
    _ORIG_COMMIT(self, inst, lazy_reg_writes)


tile.TileContext._commit_instruction = _single_wait_commit

T, B, K = 512, 512, 64
NCORES = 8
BSH = B // NCORES      # 64 batch per core
P = 16                 # real steps per chunk
M = 4                  # burn-in steps
S = T // P             # 32 chunks
LL = P + M             # 20 chain steps per chunk (zbuf rows 1..LL)
NR = LL + 1            # 21 zbuf rows (row 0 unused, kept 1.0)
NR2 = NR + 1           # 22: zT column pitch (4-byte aligned for bf16 PSUM)
NG = 4                 # groups
G = S // NG            # 8 chunks per group
GC = G * BSH           # 512 columns per group
W = 4                  # emission window (steps per DMA/exp block)
NW = LL // W           # 5 windows
DSPLIT = 320           # mult columns 0..DSPLIT-1 on DVE, rest on Pool
DELTA = 4.0            # per-step log-space offset folded into ETs
F32 = mybir.dt.float32
BF16 = mybir.dt.bfloat16
MULT = mybir.AluOpType.mult
ADD = mybir.AluOpType.add
SUB = mybir.AluOpType.subtract
AX = mybir.AxisListType.X
AF = mybir.ActivationFunctionType


def _t_start(c):
    return 0 if c == 0 else c * P - M


def _build_crf_nc() -> bass.Bass:
    nc = bass.Bass(trn_type="TRN2", target_bir_lowering=False, debug=False)

    # emissions host-prearranged into per-(group,window) staging blocks:
    # row (g*NW + wv)*K + k, col = chunk_in_group*(W*BSH) + step_in_window*BSH
    # + b.  One 2-descriptor-per-partition DMA loads half a window block.
    emt_d = nc.dram_tensor(
        "emits_blk", [NG * NW * K, G * W * BSH], BF16, kind="ExternalInput"
    ).ap()
    trans_d = nc.dram_tensor("transitions", [K, K], F32, kind="ExternalInput").ap()
    alpha0_d = nc.dram_tensor("alpha_0", [K, 1], F32, kind="ExternalInput").ap()
    ohz_d = nc.dram_tensor("onehot_z", [BSH, S * NR2], F32, kind="ExternalInput").ap()
    ohc_d = nc.dram_tensor("onehot_c", [BSH, S], F32, kind="ExternalInput").ap()
    taud_d = nc.dram_tensor("tau_delta", [BSH, 1], F32, kind="ExternalInput").ap()
    out_d = nc.dram_tensor("out_sum", [1, 1], F32, kind="ExternalOutput").ap()

    with tile.TileContext(nc) as tc:
        with ExitStack() as ctx:
            _crf_body(ctx, tc, emt_d, trans_d, alpha0_d, ohz_d, ohc_d, taud_d,
                      out_d)
    _split_remaining_multiwaits(nc)
    return nc


def _split_remaining_multiwaits(nc):
    for blk in nc.m.functions[0].blocks:
        il = blk.instructions
        idx = 0
        while idx < len(il):
            inst = il[idx]
            si = inst.sync_info
            if si is not None and si.on_wait and len(si.on_wait) > 1:
                waits = list(si.on_wait)
                for j, w in enumerate(waits[:-1]):
                    n = mybir.InstNoOp(
                        name=f"I-swx-{inst.name}-{j}", ins=[], outs=[]
                    )
                    n.engine = inst.engine
                    n.sync_info = mybir.SyncInfo(on_wait=[w], on_update=[])
                    nc.register_instruction(n, overwrite=True)
                    il.insert(idx, n)
                    idx += 1
                inst.sync_info = mybir.SyncInfo(
                    on_wait=[waits[-1]], on_update=list(si.on_update or [])
                )
            idx += 1


def _crf_body(ctx, tc, emt_d, trans_d, alpha0_d, ohz_d, ohc_d, taud_d, out_d):
    nc = tc.nc

    # ---- long-lived SBUF ----
    ets = nc.alloc_sbuf_tensor("ets", [K, K + 1], BF16).ap()
    expal = nc.alloc_sbuf_tensor("expal", [K + 1, 1], F32).ap()
    identf = nc.alloc_sbuf_tensor("identf", [NR + 1, NR + 1], BF16).ap()
    ones_b = nc.alloc_sbuf_tensor("ones_b", [BSH, 1], F32).ap()
    cst = nc.alloc_sbuf_tensor("cst", [128, 2], F32).ap()  # col0=0, col1=-DELTA
    zbuf = [
        nc.alloc_sbuf_tensor(f"zbuf{g}", [NR, GC], BF16).ap() for g in range(NG)
    ]
    # state ring: slot s holds w_s [65, GC]; row 64 = Z(s-1) passthrough,
    # harvested by one gather-DMA per group after the chain.
    wring = [
        nc.alloc_sbuf_tensor(f"wring{g}", [K + 1, (LL + 1) * GC], BF16).ap()
        for g in range(NG)
    ]
    # emission staging: per group 2 persistent buffers [65, W*GC] bf16 with
    # row 64 = 1.0 (preset once) so the 65-row multiply passes Z through.
    eexp = [
        [nc.alloc_sbuf_tensor(f"eexp{g}_{i}", [K + 1, W * GC], BF16).ap()
         for i in range(2)]
        for g in range(NG)
    ]

    # ---- one-time setup ----
    nc.gpsimd.memset(cst[:, 0:1], 0.0)
    nc.gpsimd.memset(cst[:, 1:2], -DELTA)
    nc.gpsimd.memset(ones_b[:, :], 1.0)
    for g in range(NG):
        nc.vector.memset(zbuf[g][:, :], 1.0)  # row 0 stays 1.0 -> ln = 0
        for i in range(2):
            nc.vector.memset(eexp[g][i][K : K + 1, :], 1.0)
    make_identity(nc, identf)

    fin_pool = ctx.enter_context(tc.tile_pool(name="fin", bufs=1))

    tr_t = fin_pool.tile([K, K], F32, tag="trt")
    nc.sync.dma_start(tr_t[:], trans_d)
    nc.scalar.activation(ets[:, 0:K], tr_t[:], AF.Exp, bias=cst[0:K, 1:2])
    nc.vector.memset(ets[:, K : K + 1], 1.0)

    a0_t = fin_pool.tile([K, 1], F32, tag="a0t")
    nc.sync.dma_start(a0_t[:], alpha0_d)
    nc.scalar.activation(expal[0:K], a0_t[:], AF.Exp, bias=cst[0:K, 0:1])
    nc.vector.memset(expal[K : K + 1], 1.0)

    ohz = fin_pool.tile([BSH, S * NR2], F32, tag="ohz")
    nc.sync.dma_start(ohz[:], ohz_d)
    ohc = fin_pool.tile([BSH, S], F32, tag="ohc")
    nc.sync.dma_start(ohc[:], ohc_d)
    taud = fin_pool.tile([BSH, 1], F32, tag="taud")
    nc.sync.dma_start(taud[:], taud_d)

    with ExitStack() as chain_ctx:
        raw_pool = chain_ctx.enter_context(tc.tile_pool(name="raw", bufs=2))
        u_psum = chain_ctx.enter_context(
            tc.tile_pool(name="upsum", bufs=2, space="PSUM")
        )

        # window DMA + exp: raw [64, W*GC] bf16 in the exact staging layout;
        # two big contiguous DMAs per (group, window) on alternating queues.
        def load_window(g, wv):
            rt = raw_pool.tile([K, W * GC], BF16, tag=f"raw{g}")
            r0 = (g * NW + wv) * K
            half = W * GC // 2
            nc.gpsimd.dma_start(rt[:, 0:half], emt_d[r0 : r0 + K, 0:half])
            nc.sync.dma_start(
                rt[:, half : W * GC], emt_d[r0 : r0 + K, half : W * GC]
            )
            dst = eexp[g][wv % 2]
            nc.scalar.activation(dst[0:K, :], rt[:], AF.Exp, bias=cst[0:K, 0:1])

        for g in range(NG):
            load_window(g, 0)
        for g in range(NG):
            load_window(g, 1)

        # init states: w0 = e_{t0} (chunks >=1), chunk 0: expal * e_0
        for g in range(NG):
            wt = wring[g][:, 0:GC]
            sv = eexp[g][0][:].rearrange("k (c t b) -> k c t b", c=G, t=W)
            wv_ = wt.rearrange("k (c b) -> k c b", c=G)
            if g == 0:
                nc.vector.tensor_scalar(
                    wv_[:, 0, :], sv[:, 0, 0, :], expal, None, op0=MULT
                )
                nc.vector.tensor_copy(wv_[:, 1:G, :], sv[:, 1:G, 0, :])
            else:
                nc.vector.tensor_copy(wv_[:, :, :], sv[:, :, 0, :])

        # ---- chain: steps 1..LL ----
        for s in range(1, LL + 1):
            if s % W == 0 and s // W + 1 < NW:
                for g in range(NG):
                    load_window(g, s // W + 1)
            se = min(s, LL - 1)        # step LL reuses step LL-1's emission
            wv, sw = se // W, se % W
            for g in range(NG):
                u = u_psum.tile([K + 1, GC], F32, tag=f"u{g}")
                nc.tensor.matmul(
                    u[:],
                    ets[:, :],
                    wring[g][0:K, (s - 1) * GC : s * GC],
                    start=True,
                    stop=True,
                )
                sv = eexp[g][wv % 2][:].rearrange(
                    "k (c t b) -> k c t b", c=G, t=W
                )
                nc.vector.tensor_tensor(
                    wring[g][:, s * GC : (s + 1) * GC].rearrange(
                        "k (c b) -> k c b", c=G
                    ),
                    u[:].rearrange("k (c b) -> k c b", c=G),
                    sv[:, :, sw, :],
                    op=MULT,
                )
        # Z harvest: row 64 of slots 1..LL -> zbuf rows 1..LL (one DMA/group)
        for g in range(NG):
            nc.gpsimd.dma_start(
                zbuf[g][1 : LL + 1, :],
                wring[g][K : K + 1, GC : (LL + 1) * GC].rearrange(
                    "r (s c) -> r s c", s=LL
                ),
            )

    # ---- final combine ----
    fpsum = ctx.enter_context(tc.tile_pool(name="fpsum", bufs=2, space="PSUM"))
    zT = fin_pool.tile([BSH, S * NR2], F32, tag="zT")
    # pad columns would otherwise hold junk; preset whole tile Ln-safe
    nc.vector.memset(zT[:, :], 1.0)
    for h in range(2):
        zt = fpsum.tile([BSH, (S // 2) * NR2], BF16, tag="zt")
        for ci in range(S // 2):
            c = h * (S // 2) + ci
            g, gi = c // G, c % G
            nc.tensor.transpose(
                zt[:, ci * NR2 : ci * NR2 + NR],
                zbuf[g][:, gi * BSH : (gi + 1) * BSH],
                identf[0:NR, 0:NR],
            )
        nc.vector.tensor_copy(
            zT[:, h * (S // 2) * NR2 : (h + 1) * (S // 2) * NR2].rearrange(
                "b (c r) -> b c r", r=NR2
            )[:, :, 0:NR],
            zt[:].rearrange("b (c r) -> b c r", r=NR2)[:, :, 0:NR],
        )
    # patch: chunk0's matching column (row LL) := its row P (t = P-1)
    nc.vector.tensor_copy(zT[:, LL : LL + 1], zT[:, P : P + 1])
    lnz = fin_pool.tile([BSH, S * NR2], F32, tag="lnz")
    nc.scalar.activation(lnz[:], zT[:], AF.Ln, bias=cst[0:BSH, 0:1])

    # delta stitching: inc[:, i] = lnz[:, NR*(i-1) + LL] - lnz[:, NR*i + M]
    lv = lnz[:].rearrange("b (c r) -> b c r", r=NR2)
    inc = fin_pool.tile([BSH, S], F32, tag="inc")
    nc.vector.memset(inc[:, 0:1], 0.0)
    nc.vector.tensor_tensor(
        inc[:, 1:S], lv[:, 0 : S - 1, LL], lv[:, 1:S, M], op=SUB
    )
    # cumulative sum over chunks (Hillis-Steele, ping-pong)
    cs_a = inc
    for k in (1, 2, 4, 8, 16):
        cs_b = fin_pool.tile([BSH, S], F32, tag=f"cs{k}")
        nc.vector.tensor_copy(cs_b[:, 0:k], cs_a[:, 0:k])
        nc.vector.tensor_tensor(
            cs_b[:, k:S], cs_a[:, k:S], cs_a[:, 0 : S - k], op=ADD
        )
        cs_a = cs_b

    scr1 = fin_pool.tile([BSH, S * NR2], F32, tag="scr1")
    zsel = fin_pool.tile([BSH, 1], F32, tag="zsel")
    nc.vector.tensor_tensor(scr1[:], lnz[:], ohz[:], op=MULT)
    nc.vector.tensor_reduce(zsel[:], scr1[:], axis=AX, op=ADD)
    scr2 = fin_pool.tile([BSH, S], F32, tag="scr2")
    dsel = fin_pool.tile([BSH, 1], F32, tag="dsel")
    nc.vector.tensor_tensor(scr2[:], cs_a[:], ohc[:], op=MULT)
    nc.vector.tensor_reduce(dsel[:], scr2[:], axis=AX, op=ADD)
    res = fin_pool.tile([BSH, 1], F32, tag="res")
    nc.vector.tensor_tensor(res[:], zsel[:], dsel[:], op=ADD)
    nc.vector.tensor_tensor(res[:], res[:], taud[:], op=ADD)
    acc = fpsum.tile([1, 1], F32, tag="acc")
    nc.tensor.matmul(acc[:], res[:], ones_b[:], start=True, stop=True)
    osb = fin_pool.tile([1, 1], F32, tag="osb")
    nc.scalar.copy(osb[:], acc[:])
    nc.sync.dma_start(out_d, osb[:])


_NC_CACHE = None


def _get_nc():
    global _NC_CACHE
    if _NC_CACHE is None:
        _NC_CACHE = _build_crf_nc()
    return _NC_CACHE


def _make_in_maps(np_inputs):
    emits = np.asarray(np_inputs["emits"], dtype=np.float32)
    mask = np.asarray(np_inputs["mask"])
    transitions = np.asarray(np_inputs["transitions"], dtype=np.float32)
    alpha_0 = np.asarray(np_inputs["alpha_0"], dtype=np.float32)
    emits_t = emits.transpose(0, 2, 1)  # [T, K, B] view
    tau = mask.argmax(0).astype(np.int64)  # [B]
    chunk = tau // P
    row = np.where(chunk == 0, tau + 1, tau % P + M + 1)
    in_maps = []
    for cix in range(NCORES):
        sl = slice(cix * BSH, (cix + 1) * BSH)
        tau_s, c_s, r_s = tau[sl], chunk[sl], row[sl]
        ohz = np.zeros((BSH, S * NR2), dtype=np.float32)
        ohz[np.arange(BSH), c_s * NR2 + r_s] = 1.0
        ohc = np.zeros((BSH, S), dtype=np.float32)
        ohc[np.arange(BSH), c_s] = 1.0
        taud = (DELTA * tau_s).astype(np.float32).reshape(BSH, 1)
        sh = emits_t[:, :, sl]  # [T, K, 64]
        blk = np.empty((NG, NW, K, G, W, BSH), dtype=np.float32)
        for g in range(NG):
            for ci in range(G):
                t0 = _t_start(g * G + ci)
                blk[g, :, :, ci, :, :] = (
                    sh[t0 : t0 + LL].reshape(NW, W, K, BSH).transpose(0, 2, 1, 3)
                )
        import ml_dtypes
        emb = blk.reshape(NG * NW * K, G * W * BSH).astype(ml_dtypes.bfloat16)
        in_maps.append(
            {
                "emits_blk": emb,
                "transitions": transitions,
                "alpha_0": alpha_0,
                "onehot_z": ohz,
                "onehot_c": ohc,
                "tau_delta": taud,
            }
        )
    return in_maps


def kernel(emits, mask, transitions, alpha_0):
    nc = _get_nc()
    in_maps = _make_in_maps(
        {"emits": emits, "mask": mask, "transitions": transitions,
         "alpha_0": alpha_0}
    )
    res = run_bass_kernel_spmd(nc, in_maps, core_ids=list(range(NCORES)))
    total = np.float64(0.0)
    for r in res.results:
        total += np.asarray(r["out_sum"], dtype=np.float64).sum()
    return np.float32(total)


# revision 16
# speedup vs baseline: 3.6683x; 1.2019x over previous
"""CRF forward (logsumexp over paths) loss kernel for Trainium2, 8 NeuronCores.

Time-parallel chunked algorithm
-------------------------------
The linear-space recurrence  w_t = (ETs^T w_{t-1}) * e_t  (ETs = exp(trans-D),
e_t = exp(emit_t), state [K, B] per core) is a product of positive matrices,
so it forgets its initial condition at the Birkhoff contraction rate —
measured here at ~2 decades per 2 steps.  That lets the T=512 serial chain be
cut into S=32 time chunks run CONCURRENTLY: each chunk starts from the
uniform state w := e_{t0} a couple of steps (m=2) before its real range and
is correct in *direction* by the time the range starts; its unknown per-batch
log-magnitude offset delta_c is recovered afterwards by matching log-colsums
with the previous chunk at the shared boundary step (a tiny scalar cumsum).

Per core (64-batch shard), the 32 chunks run as 2 pair-groups of 16 batched
into the free axis: two [65, 512] matmuls per pair-step (65th weight column
of ones emits the colsum row Z for free) land in one [65, 1024] PSUM tile,
consumed by a single DVE multiply whose emission operand has a preset ones
row — so Z rides through into the persistent SBUF state ring and is
harvested by ONE gather-DMA per pair after the chain (GPSIMD DMA triggers
cost ~800ns each, so DMA count is minimized everywhere: emissions are
host-prearranged into the exact staging layout and load as two big
contiguous DMAs per pair-window).  Emissions are exp'd on the Scalar engine.
Final combine: per-chunk Z histories are PE-transposed to [b, slot], matched
into delta_c (log-ratio cumsum), and the one-hot time mask (host-preprocessed
into one-hot (chunk,slot) + chunk indicators) selects
ln Z(tau_b) + delta_c(b) + D*tau_b; a ones-matmul reduces the batch on core.

Sharding: batch 512 = 8 cores x 64, transitions/alpha_0 replicated; host sums
the 8 per-core scalars.
"""

import os
import sys

for _p in ("/opt/trn_rl_repo", "/root/.axon_site/_ro/trn_rl_repo"):
    if os.path.isdir(_p) and _p not in sys.path:
        sys.path.insert(0, _p)

from contextlib import ExitStack

import numpy as np

import concourse.bass as bass
import concourse.mybir as mybir
import concourse.tile as tile
from concourse.bass_utils import run_bass_kernel_spmd
from concourse.masks import make_identity

# Walrus in this container rejects instructions with >1 sync-wait; split the
# extras onto preceding same-engine no-ops (queues are in-order, so identical
# semantics).
_ORIG_COMMIT = tile.TileContext._commit_instruction


def _single_wait_commit(self, inst, lazy_reg_writes=True):
    si = getattr(inst, "sync_info", None)
    if (
        si is not None
        and si.on_wait
        and len(si.on_wait) > 1
        and inst.engine != mybir.EngineType.Unassigned
    ):
        waits = list(si.on_wait)
        eng = self.nc.engines[inst.engine]
        for w in waits[:-1]:
            n = eng.nop(nofuse=True)
            n.ins.sync_info = mybir.SyncInfo(on_wait=[w], on_update=[])
        inst.sync_info = mybir.SyncInfo(
            on_wait=[waits[-1]], on_update=list(si.on_update or [])
        )
    _ORIG_COMMIT(self, inst, lazy_reg_writes)


tile.TileContext._commit_instruction = _single_wait_commit

T, B, K = 512, 512, 64
NCORES = 8
BSH = B // NCORES      # 64 batch per core
P = 16                 # real steps per chunk
M = 2                  # burn-in steps
S = T // P             # 32 chunks
LL = P + M             # 18 chain steps per chunk (zbuf rows 1..LL)
NR = LL + 1            # 19 zbuf rows (row 0 unused, kept 1.0)
NR2 = NR + 1           # 20: zT column pitch (4-byte aligned for bf16 PSUM)
NP = 2                 # pair-groups
GP = S // NP           # 16 chunks per pair-group
PC = GP * BSH          # 1024 columns per pair-group
HC = PC // 2           # 512 columns per matmul
W = 3                  # emission window (steps per DMA/exp block)
NW = LL // W           # 6 windows
DELTA = 4.0            # per-step log-space offset folded into ETs
F32 = mybir.dt.float32
BF16 = mybir.dt.bfloat16
MULT = mybir.AluOpType.mult
ADD = mybir.AluOpType.add
SUB = mybir.AluOpType.subtract
AX = mybir.AxisListType.X
AF = mybir.ActivationFunctionType


def _t_start(c):
    return 0 if c == 0 else c * P - M


def _build_crf_nc() -> bass.Bass:
    nc = bass.Bass(trn_type="TRN2", target_bir_lowering=False, debug=False)

    # emissions host-prearranged into per-(pair,window) staging blocks:
    # row (p*NW + wv)*K + k, col = step_in_window*PC + chunk_in_pair*BSH + b
    emt_d = nc.dram_tensor(
        "emits_blk", [NP * NW * K, W * PC], BF16, kind="ExternalInput"
    ).ap()
    trans_d = nc.dram_tensor("transitions", [K, K], F32, kind="ExternalInput").ap()
    alpha0_d = nc.dram_tensor("alpha_0", [K, 1], F32, kind="ExternalInput").ap()
    ohz_d = nc.dram_tensor("onehot_z", [BSH, S * NR2], F32, kind="ExternalInput").ap()
    ohc_d = nc.dram_tensor("onehot_c", [BSH, S], F32, kind="ExternalInput").ap()
    taud_d = nc.dram_tensor("tau_delta", [BSH, 1], F32, kind="ExternalInput").ap()
    out_d = nc.dram_tensor("out_sum", [1, 1], F32, kind="ExternalOutput").ap()

    with tile.TileContext(nc) as tc:
        with ExitStack() as ctx:
            _crf_body(ctx, tc, emt_d, trans_d, alpha0_d, ohz_d, ohc_d, taud_d,
                      out_d)
    _split_remaining_multiwaits(nc)
    return nc


def _split_remaining_multiwaits(nc):
    for blk in nc.m.functions[0].blocks:
        il = blk.instructions
        idx = 0
        while idx < len(il):
            inst = il[idx]
            si = inst.sync_info
            if si is not None and si.on_wait and len(si.on_wait) > 1:
                waits = list(si.on_wait)
                for j, w in enumerate(waits[:-1]):
                    n = mybir.InstNoOp(
                        name=f"I-swx-{inst.name}-{j}", ins=[], outs=[]
                    )
                    n.engine = inst.engine
                    n.sync_info = mybir.SyncInfo(on_wait=[w], on_update=[])
                    nc.register_instruction(n, overwrite=True)
                    il.insert(idx, n)
                    idx += 1
                inst.sync_info = mybir.SyncInfo(
                    on_wait=[waits[-1]], on_update=list(si.on_update or [])
                )
            idx += 1


def _crf_body(ctx, tc, emt_d, trans_d, alpha0_d, ohz_d, ohc_d, taud_d, out_d):
    nc = tc.nc

    # ---- long-lived SBUF ----
    ets = nc.alloc_sbuf_tensor("ets", [K, K + 1], BF16).ap()
    expal = nc.alloc_sbuf_tensor("expal", [K + 1, 1], F32).ap()
    identf = nc.alloc_sbuf_tensor("identf", [NR + 1, NR + 1], BF16).ap()
    ones_b = nc.alloc_sbuf_tensor("ones_b", [BSH, 1], F32).ap()
    cst = nc.alloc_sbuf_tensor("cst", [128, 2], F32).ap()  # col0=0, col1=-DELTA
    zbuf = [
        nc.alloc_sbuf_tensor(f"zbuf{p}", [NR, PC], BF16).ap() for p in range(NP)
    ]
    # state ring: slot s holds w_s [65, PC]; row 64 = Z(s-1) passthrough,
    # harvested by one gather-DMA per pair after the chain.
    wring = [
        nc.alloc_sbuf_tensor(f"wring{p}", [K + 1, (LL + 1) * PC], BF16).ap()
        for p in range(NP)
    ]
    # emission staging: per pair 3 persistent buffers [65, W*PC] bf16 with
    # row 64 = 1.0 (preset once) so the 65-row multiply passes Z through.
    eexp = [
        [nc.alloc_sbuf_tensor(f"eexp{p}_{i}", [K + 1, W * PC], BF16).ap()
         for i in range(3)]
        for p in range(NP)
    ]

    # ---- one-time setup (gpsimd: keeps the DVE queue free at startup) ----
    nc.gpsimd.memset(cst[:, 0:1], 0.0)
    nc.gpsimd.memset(cst[:, 1:2], -DELTA)
    nc.gpsimd.memset(ones_b[:, :], 1.0)
    for p in range(NP):
        nc.gpsimd.memset(zbuf[p][:, :], 1.0)  # row 0 stays 1.0 -> ln = 0
        for i in range(3):
            nc.gpsimd.memset(eexp[p][i][K : K + 1, :], 1.0)
    make_identity(nc, identf)

    fin_pool = ctx.enter_context(tc.tile_pool(name="fin", bufs=1))

    tr_t = fin_pool.tile([K, K], F32, tag="trt")
    nc.sync.dma_start(tr_t[:], trans_d)
    nc.scalar.activation(ets[:, 0:K], tr_t[:], AF.Exp, bias=cst[0:K, 1:2])
    nc.vector.memset(ets[:, K : K + 1], 1.0)

    a0_t = fin_pool.tile([K, 1], F32, tag="a0t")
    nc.sync.dma_start(a0_t[:], alpha0_d)
    nc.scalar.activation(expal[0:K], a0_t[:], AF.Exp, bias=cst[0:K, 0:1])
    nc.vector.memset(expal[K : K + 1], 1.0)

    ohz = fin_pool.tile([BSH, S * NR2], F32, tag="ohz")
    nc.sync.dma_start(ohz[:], ohz_d)
    ohc = fin_pool.tile([BSH, S], F32, tag="ohc")
    nc.sync.dma_start(ohc[:], ohc_d)
    taud = fin_pool.tile([BSH, 1], F32, tag="taud")
    nc.sync.dma_start(taud[:], taud_d)

    with ExitStack() as chain_ctx:
        raw_pool = chain_ctx.enter_context(tc.tile_pool(name="raw", bufs=3))
        u_psum = chain_ctx.enter_context(
            tc.tile_pool(name="upsum", bufs=2, space="PSUM")
        )

        def load_window(p, wv):
            rt = raw_pool.tile([K, W * PC], BF16, tag=f"raw{p}")
            r0 = (p * NW + wv) * K
            half = W * PC // 2
            nc.gpsimd.dma_start(rt[:, 0:half], emt_d[r0 : r0 + K, 0:half])
            nc.sync.dma_start(
                rt[:, half : W * PC], emt_d[r0 : r0 + K, half : W * PC]
            )
            dst = eexp[p][wv % 3]
            nc.scalar.activation(dst[0:K, :], rt[:], AF.Exp, bias=cst[0:K, 0:1])

        for wv in range(3):
            for p in range(NP):
                load_window(p, wv)

        # init states: w0 = e_{t0} (chunks >=1), chunk 0: expal * e_0
        for p in range(NP):
            wt = wring[p][:, 0:PC]
            sv = eexp[p][0][:, 0:PC]
            if p == 0:
                nc.vector.tensor_scalar(
                    wt[:, 0:BSH], sv[:, 0:BSH], expal, None, op0=MULT
                )
                nc.vector.tensor_copy(wt[:, BSH:PC], sv[:, BSH:PC])
            else:
                nc.vector.tensor_copy(wt[:, :], sv[:, :])

        # ---- chain: steps 1..LL ----
        for s in range(1, LL + 1):
            if s % W == 0 and s // W + 3 <= NW:
                for p in range(NP):
                    load_window(p, s // W + 2)
            se = min(s, LL - 1)        # step LL reuses step LL-1's emission
            wv, sw = se // W, se % W
            for p in range(NP):
                u = u_psum.tile([K + 1, PC], F32, tag=f"u{p}")
                nc.tensor.matmul(
                    u[:, 0:HC],
                    ets[:, :],
                    wring[p][0:K, (s - 1) * PC : (s - 1) * PC + HC],
                    start=True,
                    stop=True,
                )
                nc.tensor.matmul(
                    u[:, HC:PC],
                    ets[:, :],
                    wring[p][0:K, (s - 1) * PC + HC : s * PC],
                    start=True,
                    stop=True,
                )
                nc.vector.tensor_tensor(
                    wring[p][:, s * PC : (s + 1) * PC],
                    u[:, :],
                    eexp[p][wv % 3][:, sw * PC : (sw + 1) * PC],
                    op=MULT,
                )
        # Z harvest: row 64 of slots 1..LL -> zbuf rows 1..LL (one DMA/pair)
        for p in range(NP):
            nc.gpsimd.dma_start(
                zbuf[p][1 : LL + 1, :],
                wring[p][K : K + 1, PC : (LL + 1) * PC].rearrange(
                    "r (s c) -> r s c", s=LL
                ),
            )

    # ---- final combine ----
    fpsum = ctx.enter_context(tc.tile_pool(name="fpsum", bufs=2, space="PSUM"))
    zT = fin_pool.tile([BSH, S * NR2], F32, tag="zT")
    # pad columns would otherwise hold junk; preset whole tile Ln-safe
    nc.vector.memset(zT[:, :], 1.0)
    for h in range(2):
        zt = fpsum.tile([BSH, (S // 2) * NR2], BF16, tag="zt")
        for ci in range(S // 2):
            c = h * (S // 2) + ci
            p, gi = c // GP, c % GP
            nc.tensor.transpose(
                zt[:, ci * NR2 : ci * NR2 + NR],
                zbuf[p][:, gi * BSH : (gi + 1) * BSH],
                identf[0:NR, 0:NR],
            )
        nc.vector.tensor_copy(
            zT[:, h * (S // 2) * NR2 : (h + 1) * (S // 2) * NR2].rearrange(
                "b (c r) -> b c r", r=NR2
            )[:, :, 0:NR],
            zt[:].rearrange("b (c r) -> b c r", r=NR2)[:, :, 0:NR],
        )
    # patch: chunk0's matching column (row LL) := its row P (t = P-1)
    nc.vector.tensor_copy(zT[:, LL : LL + 1], zT[:, P : P + 1])
    lnz = fin_pool.tile([BSH, S * NR2], F32, tag="lnz")
    nc.scalar.activation(lnz[:], zT[:], AF.Ln, bias=cst[0:BSH, 0:1])

    # delta stitching: inc[:, i] = lnz[:, NR2*(i-1) + LL] - lnz[:, NR2*i + M]
    lv = lnz[:].rearrange("b (c r) -> b c r", r=NR2)
    inc = fin_pool.tile([BSH, S], F32, tag="inc")
    nc.vector.memset(inc[:, 0:1], 0.0)
    nc.vector.tensor_tensor(
        inc[:, 1:S], lv[:, 0 : S - 1, LL], lv[:, 1:S, M], op=SUB
    )
    # cumulative sum over chunks (Hillis-Steele, ping-pong)
    cs_a = inc
    for k in (1, 2, 4, 8, 16):
        cs_b = fin_pool.tile([BSH, S], F32, tag=f"cs{k}")
        nc.vector.tensor_copy(cs_b[:, 0:k], cs_a[:, 0:k])
        nc.vector.tensor_tensor(
            cs_b[:, k:S], cs_a[:, k:S], cs_a[:, 0 : S - k], op=ADD
        )
        cs_a = cs_b

    scr1 = fin_pool.tile([BSH, S * NR2], F32, tag="scr1")
    zsel = fin_pool.tile([BSH, 1], F32, tag="zsel")
    nc.vector.tensor_tensor(scr1[:], lnz[:], ohz[:], op=MULT)
    nc.vector.tensor_reduce(zsel[:], scr1[:], axis=AX, op=ADD)
    scr2 = fin_pool.tile([BSH, S], F32, tag="scr2")
    dsel = fin_pool.tile([BSH, 1], F32, tag="dsel")
    nc.vector.tensor_tensor(scr2[:], cs_a[:], ohc[:], op=MULT)
    nc.vector.tensor_reduce(dsel[:], scr2[:], axis=AX, op=ADD)
    res = fin_pool.tile([BSH, 1], F32, tag="res")
    nc.vector.tensor_tensor(res[:], zsel[:], dsel[:], op=ADD)
    nc.vector.tensor_tensor(res[:], res[:], taud[:], op=ADD)
    acc = fpsum.tile([1, 1], F32, tag="acc")
    nc.tensor.matmul(acc[:], res[:], ones_b[:], start=True, stop=True)
    osb = fin_pool.tile([1, 1], F32, tag="osb")
    nc.scalar.copy(osb[:], acc[:])
    nc.sync.dma_start(out_d, osb[:])


_NC_CACHE = None


def _get_nc():
    global _NC_CACHE
    if _NC_CACHE is None:
        _NC_CACHE = _build_crf_nc()
    return _NC_CACHE


def _make_in_maps(np_inputs):
    import ml_dtypes

    emits = np.asarray(np_inputs["emits"], dtype=np.float32)
    mask = np.asarray(np_inputs["mask"])
    transitions = np.asarray(np_inputs["transitions"], dtype=np.float32)
    alpha_0 = np.asarray(np_inputs["alpha_0"], dtype=np.float32)
    emits_t = emits.transpose(0, 2, 1)  # [T, K, B] view
    tau = mask.argmax(0).astype(np.int64)  # [B]
    chunk = tau // P
    row = np.where(chunk == 0, tau + 1, tau % P + M + 1)
    in_maps = []
    for cix in range(NCORES):
        sl = slice(cix * BSH, (cix + 1) * BSH)
        tau_s, c_s, r_s = tau[sl], chunk[sl], row[sl]
        ohz = np.zeros((BSH, S * NR2), dtype=np.float32)
        ohz[np.arange(BSH), c_s * NR2 + r_s] = 1.0
        ohc = np.zeros((BSH, S), dtype=np.float32)
        ohc[np.arange(BSH), c_s] = 1.0
        taud = (DELTA * tau_s).astype(np.float32).reshape(BSH, 1)
        sh = emits_t[:, :, sl]  # [T, K, 64]
        # staging blocks [pair, window, k, step_in_window, chunk_in_pair, b]
        blk = np.empty((NP, NW, K, W, GP, BSH), dtype=np.float32)
        for p in range(NP):
            for ci in range(GP):
                t0 = _t_start(p * GP + ci)
                blk[p, :, :, :, ci, :] = (
                    sh[t0 : t0 + LL].reshape(NW, W, K, BSH).transpose(0, 2, 1, 3)
                )
        emb = blk.reshape(NP * NW * K, W * PC).astype(ml_dtypes.bfloat16)
        in_maps.append(
            {
                "emits_blk": emb,
                "transitions": transitions,
                "alpha_0": alpha_0,
                "onehot_z": ohz,
                "onehot_c": ohc,
                "tau_delta": taud,
            }
        )
    return in_maps


def kernel(emits, mask, transitions, alpha_0):
    nc = _get_nc()
    in_maps = _make_in_maps(
        {"emits": emits, "mask": mask, "transitions": transitions,
         "alpha_0": alpha_0}
    )
    res = run_bass_kernel_spmd(nc, in_maps, core_ids=list(range(NCORES)))
    total = np.float64(0.0)
    for r in res.results:
        total += np.asarray(r["out_sum"], dtype=np.float64).sum()
    return np.float32(total)


# revision 17
# speedup vs baseline: 4.3801x; 1.1941x over previous
"""CRF forward (logsumexp over paths) loss kernel for Trainium2, 8 NeuronCores.

Time-parallel chunked algorithm
-------------------------------
The linear-space recurrence  w_t = (ETs^T w_{t-1}) * e_t  (ETs = exp(trans-D),
e_t = exp(emit_t), state [K, B] per core) is a product of positive matrices,
so it forgets its initial condition at the Birkhoff contraction rate —
measured here at ~2 decades per 2 steps.  That lets the T=512 serial chain be
cut into S=32 time chunks run CONCURRENTLY: each chunk starts from the
uniform state w := e_{t0} a couple of steps (m=2) before its real range and
is correct in *direction* by the time the range starts; its unknown per-batch
log-magnitude offset delta_c is recovered afterwards by matching log-colsums
with the previous chunk at the shared boundary step (a tiny scalar cumsum).

Per core (64-batch shard), the 32 chunks run as 2 pair-groups of 16 batched
into the free axis: two [65, 512] matmuls per pair-step (65th weight column
of ones emits the colsum row Z for free) land in one [65, 1024] PSUM tile,
consumed by a single DVE multiply whose emission operand has a preset ones
row — so Z rides through into the persistent SBUF state ring and is
harvested by ONE gather-DMA per pair after the chain (GPSIMD DMA triggers
cost ~800ns each, so DMA count is minimized everywhere: emissions are
host-prearranged into the exact staging layout and load as two big
contiguous DMAs per pair-window).  Emissions are exp'd on the Scalar engine.
Final combine: per-chunk Z histories are PE-transposed to [b, slot], matched
into delta_c (log-ratio cumsum), and the one-hot time mask (host-preprocessed
into one-hot (chunk,slot) + chunk indicators) selects
ln Z(tau_b) + delta_c(b) + D*tau_b; a ones-matmul reduces the batch on core.

Sharding: batch 512 = 8 cores x 64, transitions/alpha_0 replicated; host sums
the 8 per-core scalars.
"""

import os
import sys

for _p in ("/opt/trn_rl_repo", "/root/.axon_site/_ro/trn_rl_repo"):
    if os.path.isdir(_p) and _p not in sys.path:
        sys.path.insert(0, _p)

from contextlib import ExitStack

import numpy as np

import concourse.bass as bass
import concourse.mybir as mybir
import concourse.tile as tile
from concourse.bass_utils import run_bass_kernel_spmd
from concourse.masks import make_identity

# Walrus in this container rejects instructions with >1 sync-wait; split the
# extras onto preceding same-engine no-ops (queues are in-order, so identical
# semantics).
_ORIG_COMMIT = tile.TileContext._commit_instruction


def _single_wait_commit(self, inst, lazy_reg_writes=True):
    si = getattr(inst, "sync_info", None)
    if (
        si is not None
        and si.on_wait
        and len(si.on_wait) > 1
        and inst.engine != mybir.EngineType.Unassigned
    ):
        waits = list(si.on_wait)
        eng = self.nc.engines[inst.engine]
        for w in waits[:-1]:
            n = eng.nop(nofuse=True)
            n.ins.sync_info = mybir.SyncInfo(on_wait=[w], on_update=[])
        inst.sync_info = mybir.SyncInfo(
            on_wait=[waits[-1]], on_update=list(si.on_update or [])
        )
    _ORIG_COMMIT(self, inst, lazy_reg_writes)


tile.TileContext._commit_instruction = _single_wait_commit

T, B, K = 512, 512, 64
NCORES = 8
BSH = B // NCORES      # 64 batch per core
P = 16                 # real steps per chunk
M = 1                  # burn-in steps
S = T // P             # 32 chunks
LL = P + M             # 17 chain steps per chunk (zbuf rows 1..LL)
NR = LL + 1            # 18 zbuf rows (row 0 unused, kept 1.0)
NR2 = NR              # 18: zT column pitch (even -> 4-byte aligned bf16 PSUM)
NP = 2                 # pair-groups
GP = S // NP           # 16 chunks per pair-group
PC = GP * BSH          # 1024 columns per pair-group
HC = PC // 2           # 512 columns per matmul
W = 3                  # emission window (steps per DMA/exp block)
NW = 6                 # windows cover slots 0..17 (slot 17 is zero padding)
DELTA = 4.0            # per-step log-space offset folded into ETs
F32 = mybir.dt.float32
BF16 = mybir.dt.bfloat16
MULT = mybir.AluOpType.mult
ADD = mybir.AluOpType.add
SUB = mybir.AluOpType.subtract
AX = mybir.AxisListType.X
AF = mybir.ActivationFunctionType


def _t_start(c):
    return 0 if c == 0 else c * P - M


def _build_crf_nc() -> bass.Bass:
    nc = bass.Bass(trn_type="TRN2", target_bir_lowering=False, debug=False)

    # emissions host-prearranged into per-(pair,window) staging blocks:
    # row (p*NW + wv)*K + k, col = step_in_window*PC + chunk_in_pair*BSH + b
    # 65th row is 0.0 so exp() yields the ones row for Z passthrough
    emt_d = nc.dram_tensor(
        "emits_blk", [NP * NW * (K + 1), W * PC], BF16, kind="ExternalInput"
    ).ap()
    trans_d = nc.dram_tensor("transitions", [K, K], F32, kind="ExternalInput").ap()
    alpha0_d = nc.dram_tensor("alpha_0", [K, 1], F32, kind="ExternalInput").ap()
    ohz_d = nc.dram_tensor("onehot_z", [BSH, S * NR2], F32, kind="ExternalInput").ap()
    ohc_d = nc.dram_tensor("onehot_c", [BSH, S], F32, kind="ExternalInput").ap()
    taud_d = nc.dram_tensor("tau_delta", [BSH, 1], F32, kind="ExternalInput").ap()
    out_d = nc.dram_tensor("out_sum", [1, 1], F32, kind="ExternalOutput").ap()

    with tile.TileContext(nc) as tc:
        with ExitStack() as ctx:
            _crf_body(ctx, tc, emt_d, trans_d, alpha0_d, ohz_d, ohc_d, taud_d,
                      out_d)
    _split_remaining_multiwaits(nc)
    return nc


def _split_remaining_multiwaits(nc):
    for blk in nc.m.functions[0].blocks:
        il = blk.instructions
        idx = 0
        while idx < len(il):
            inst = il[idx]
            si = inst.sync_info
            if si is not None and si.on_wait and len(si.on_wait) > 1:
                waits = list(si.on_wait)
                for j, w in enumerate(waits[:-1]):
                    n = mybir.InstNoOp(
                        name=f"I-swx-{inst.name}-{j}", ins=[], outs=[]
                    )
                    n.engine = inst.engine
                    n.sync_info = mybir.SyncInfo(on_wait=[w], on_update=[])
                    nc.register_instruction(n, overwrite=True)
                    il.insert(idx, n)
                    idx += 1
                inst.sync_info = mybir.SyncInfo(
                    on_wait=[waits[-1]], on_update=list(si.on_update or [])
                )
            idx += 1


def _crf_body(ctx, tc, emt_d, trans_d, alpha0_d, ohz_d, ohc_d, taud_d, out_d):
    nc = tc.nc

    # ---- long-lived SBUF ----
    ets = nc.alloc_sbuf_tensor("ets", [K, K + 1], BF16).ap()
    expal = nc.alloc_sbuf_tensor("expal", [K + 1, 1], F32).ap()
    identf = nc.alloc_sbuf_tensor("identf", [NR + 1, NR + 1], BF16).ap()
    ones_b = nc.alloc_sbuf_tensor("ones_b", [BSH, 1], F32).ap()
    cst = nc.alloc_sbuf_tensor("cst", [128, 2], F32).ap()  # col0=0, col1=-DELTA
    zbuf = [
        nc.alloc_sbuf_tensor(f"zbuf{p}", [NR, PC], BF16).ap() for p in range(NP)
    ]
    # state ring: slot s holds w_s [65, PC]; row 64 = Z(s-1) passthrough,
    # harvested by one gather-DMA per pair after the chain.
    wring = [
        nc.alloc_sbuf_tensor(f"wring{p}", [K + 1, (LL + 1) * PC], BF16).ap()
        for p in range(NP)
    ]
    # emission staging: per pair 3 persistent buffers [65, W*PC] bf16 with
    # row 64 = 1.0 (preset once) so the 65-row multiply passes Z through.
    eexp = [
        [nc.alloc_sbuf_tensor(f"eexp{p}_{i}", [K + 1, W * PC], BF16).ap()
         for i in range(3)]
        for p in range(NP)
    ]

    # ---- one-time setup (gpsimd: keeps the DVE queue free at startup) ----
    nc.gpsimd.memset(cst[:, 0:1], 0.0)
    nc.gpsimd.memset(cst[:, 1:2], -DELTA)
    nc.gpsimd.memset(ones_b[:, :], 1.0)
    fin_pool = ctx.enter_context(tc.tile_pool(name="fin", bufs=1))

    tr_t = fin_pool.tile([K, K], F32, tag="trt")
    nc.sync.dma_start(tr_t[:], trans_d)
    nc.scalar.activation(ets[:, 0:K], tr_t[:], AF.Exp, bias=cst[0:K, 1:2])
    nc.vector.memset(ets[:, K : K + 1], 1.0)

    a0_t = fin_pool.tile([K, 1], F32, tag="a0t")
    nc.sync.dma_start(a0_t[:], alpha0_d)
    nc.scalar.activation(expal[0:K], a0_t[:], AF.Exp, bias=cst[0:K, 0:1])
    nc.vector.memset(expal[K : K + 1], 1.0)

    ohz = fin_pool.tile([BSH, S * NR2], F32, tag="ohz")
    nc.sync.dma_start(ohz[:], ohz_d)
    ohc = fin_pool.tile([BSH, S], F32, tag="ohc")
    nc.sync.dma_start(ohc[:], ohc_d)
    taud = fin_pool.tile([BSH, 1], F32, tag="taud")
    nc.sync.dma_start(taud[:], taud_d)

    with ExitStack() as chain_ctx:
        raw_pool = chain_ctx.enter_context(tc.tile_pool(name="raw", bufs=3))
        u_psum = chain_ctx.enter_context(
            tc.tile_pool(name="upsum", bufs=2, space="PSUM")
        )

        def load_window(p, wv):
            rt = raw_pool.tile([K + 1, W * PC], BF16, tag=f"raw{p}")
            r0 = (p * NW + wv) * (K + 1)
            q = W * PC // 4
            for i in range(4):
                eng = nc.gpsimd if i % 2 == 0 else nc.sync
                eng.dma_start(
                    rt[:, i * q : (i + 1) * q],
                    emt_d[r0 : r0 + K + 1, i * q : (i + 1) * q],
                )
            dst = eexp[p][wv % 3]
            nc.scalar.activation(dst[:, :], rt[:], AF.Exp, bias=cst[0 : K + 1, 0:1])

        for wv in range(3):
            for p in range(NP):
                load_window(p, wv)
        # bulky one-time setup AFTER the first loads so it never delays them
        for p in range(NP):
            nc.gpsimd.memset(zbuf[p][0:1, :], 1.0)  # row 0 -> ln = 0
        make_identity(nc, identf)

        # init states: w0 = e_{t0} (chunks >=1), chunk 0: expal * e_0
        for p in range(NP):
            wt = wring[p][:, 0:PC]
            sv = eexp[p][0][:, 0:PC]
            if p == 0:
                nc.vector.tensor_scalar(
                    wt[:, 0:BSH], sv[:, 0:BSH], expal, None, op0=MULT
                )
                nc.vector.tensor_copy(wt[:, BSH:PC], sv[:, BSH:PC])
            else:
                nc.vector.tensor_copy(wt[:, :], sv[:, :])

        # ---- chain: steps 1..LL ----
        for s in range(1, LL + 1):
            if s % W == 0 and s // W + 3 <= NW:
                for p in range(NP):
                    load_window(p, s // W + 2)
            se = min(s, LL - 1)        # step LL reuses step LL-1's emission
            wv, sw = se // W, se % W
            for p in range(NP):
                u = u_psum.tile([K + 1, PC], F32, tag=f"u{p}")
                nc.tensor.matmul(
                    u[:, 0:HC],
                    ets[:, :],
                    wring[p][0:K, (s - 1) * PC : (s - 1) * PC + HC],
                    start=True,
                    stop=True,
                )
                nc.tensor.matmul(
                    u[:, HC:PC],
                    ets[:, :],
                    wring[p][0:K, (s - 1) * PC + HC : s * PC],
                    start=True,
                    stop=True,
                )
                nc.vector.tensor_tensor(
                    wring[p][:, s * PC : (s + 1) * PC],
                    u[:, :],
                    eexp[p][wv % 3][:, sw * PC : (sw + 1) * PC],
                    op=MULT,
                )
        # Z harvest: row 64 of slots 1..LL -> zbuf rows 1..LL (one DMA/pair)
        for p in range(NP):
            nc.gpsimd.dma_start(
                zbuf[p][1 : LL + 1, :],
                wring[p][K : K + 1, PC : (LL + 1) * PC].rearrange(
                    "r (s c) -> r s c", s=LL
                ),
            )

    # ---- final combine ----
    fpsum = ctx.enter_context(tc.tile_pool(name="fpsum", bufs=2, space="PSUM"))
    zT = fin_pool.tile([BSH, S * NR2], F32, tag="zT")
    # pad columns would otherwise hold junk; preset whole tile Ln-safe
    nc.vector.memset(zT[:, :], 1.0)
    for h in range(2):
        zt = fpsum.tile([BSH, (S // 2) * NR2], BF16, tag="zt")
        for ci in range(S // 2):
            c = h * (S // 2) + ci
            p, gi = c // GP, c % GP
            nc.tensor.transpose(
                zt[:, ci * NR2 : ci * NR2 + NR],
                zbuf[p][:, gi * BSH : (gi + 1) * BSH],
                identf[0:NR, 0:NR],
            )
        nc.vector.tensor_copy(
            zT[:, h * (S // 2) * NR2 : (h + 1) * (S // 2) * NR2].rearrange(
                "b (c r) -> b c r", r=NR2
            )[:, :, 0:NR],
            zt[:].rearrange("b (c r) -> b c r", r=NR2)[:, :, 0:NR],
        )
    # patch: chunk0's matching column (row LL) := its row P (t = P-1)
    nc.vector.tensor_copy(zT[:, LL : LL + 1], zT[:, P : P + 1])
    lnz = fin_pool.tile([BSH, S * NR2], F32, tag="lnz")
    nc.scalar.activation(lnz[:], zT[:], AF.Ln, bias=cst[0:BSH, 0:1])

    # delta stitching: inc[:, i] = lnz[:, NR2*(i-1) + LL] - lnz[:, NR2*i + M]
    lv = lnz[:].rearrange("b (c r) -> b c r", r=NR2)
    inc = fin_pool.tile([BSH, S], F32, tag="inc")
    nc.vector.memset(inc[:, 0:1], 0.0)
    nc.vector.tensor_tensor(
        inc[:, 1:S], lv[:, 0 : S - 1, LL], lv[:, 1:S, M], op=SUB
    )
    # cumulative sum over chunks (Hillis-Steele, ping-pong)
    cs_a = inc
    for k in (1, 2, 4, 8, 16):
        cs_b = fin_pool.tile([BSH, S], F32, tag=f"cs{k}")
        nc.vector.tensor_copy(cs_b[:, 0:k], cs_a[:, 0:k])
        nc.vector.tensor_tensor(
            cs_b[:, k:S], cs_a[:, k:S], cs_a[:, 0 : S - k], op=ADD
        )
        cs_a = cs_b

    scr1 = fin_pool.tile([BSH, S * NR2], F32, tag="scr1")
    zsel = fin_pool.tile([BSH, 1], F32, tag="zsel")
    nc.vector.tensor_tensor(scr1[:], lnz[:], ohz[:], op=MULT)
    nc.vector.tensor_reduce(zsel[:], scr1[:], axis=AX, op=ADD)
    scr2 = fin_pool.tile([BSH, S], F32, tag="scr2")
    dsel = fin_pool.tile([BSH, 1], F32, tag="dsel")
    nc.vector.tensor_tensor(scr2[:], cs_a[:], ohc[:], op=MULT)
    nc.vector.tensor_reduce(dsel[:], scr2[:], axis=AX, op=ADD)
    res = fin_pool.tile([BSH, 1], F32, tag="res")
    nc.vector.tensor_tensor(res[:], zsel[:], dsel[:], op=ADD)
    nc.vector.tensor_tensor(res[:], res[:], taud[:], op=ADD)
    acc = fpsum.tile([1, 1], F32, tag="acc")
    nc.tensor.matmul(acc[:], res[:], ones_b[:], start=True, stop=True)
    osb = fin_pool.tile([1, 1], F32, tag="osb")
    nc.scalar.copy(osb[:], acc[:])
    nc.sync.dma_start(out_d, osb[:])


_NC_CACHE = None


def _get_nc():
    global _NC_CACHE
    if _NC_CACHE is None:
        _NC_CACHE = _build_crf_nc()
    return _NC_CACHE


def _make_in_maps(np_inputs):
    import ml_dtypes

    emits = np.asarray(np_inputs["emits"], dtype=np.float32)
    mask = np.asarray(np_inputs["mask"])
    transitions = np.asarray(np_inputs["transitions"], dtype=np.float32)
    alpha_0 = np.asarray(np_inputs["alpha_0"], dtype=np.float32)
    emits_t = emits.transpose(0, 2, 1)  # [T, K, B] view
    tau = mask.argmax(0).astype(np.int64)  # [B]
    chunk = tau // P
    row = np.where(chunk == 0, tau + 1, tau % P + M + 1)
    in_maps = []
    for cix in range(NCORES):
        sl = slice(cix * BSH, (cix + 1) * BSH)
        tau_s, c_s, r_s = tau[sl], chunk[sl], row[sl]
        ohz = np.zeros((BSH, S * NR2), dtype=np.float32)
        ohz[np.arange(BSH), c_s * NR2 + r_s] = 1.0
        ohc = np.zeros((BSH, S), dtype=np.float32)
        ohc[np.arange(BSH), c_s] = 1.0
        taud = (DELTA * tau_s).astype(np.float32).reshape(BSH, 1)
        sh = emits_t[:, :, sl]  # [T, K, 64]
        # staging blocks [pair, window, k(+zero row), step, chunk_in_pair, b]
        nslot = NW * W
        blk = np.zeros((NP, NW, K + 1, W, GP, BSH), dtype=np.float32)
        for p in range(NP):
            for ci in range(GP):
                t0 = _t_start(p * GP + ci)
                ns = min(nslot, T - t0)
                sv = np.zeros((nslot, K, BSH), dtype=np.float32)
                sv[:ns] = sh[t0 : t0 + ns]
                blk[p, :, 0:K, :, ci, :] = (
                    sv.reshape(NW, W, K, BSH).transpose(0, 2, 1, 3)
                )
        emb = blk.reshape(NP * NW * (K + 1), W * PC).astype(ml_dtypes.bfloat16)
        in_maps.append(
            {
                "emits_blk": emb,
                "transitions": transitions,
                "alpha_0": alpha_0,
                "onehot_z": ohz,
                "onehot_c": ohc,
                "tau_delta": taud,
            }
        )
    return in_maps


def kernel(emits, mask, transitions, alpha_0):
    nc = _get_nc()
    in_maps = _make_in_maps(
        {"emits": emits, "mask": mask, "transitions": transitions,
         "alpha_0": alpha_0}
    )
    res = run_bass_kernel_spmd(nc, in_maps, core_ids=list(range(NCORES)))
    total = np.float64(0.0)
    for r in res.results:
        total += np.asarray(r["out_sum"], dtype=np.float64).sum()
    return np.float32(total)


# revision 20
# speedup vs baseline: 4.4918x; 1.0255x over previous
"""CRF forward (logsumexp over paths) loss kernel for Trainium2, 8 NeuronCores.

Time-parallel chunked algorithm
-------------------------------
The linear-space recurrence  w_t = (ETs^T w_{t-1}) * e_t  (ETs = exp(trans-D),
e_t = exp(emit_t), state [K, B] per core) is a product of positive matrices,
so it forgets its initial condition at the Birkhoff contraction rate —
measured here at ~2 decades per 2 steps.  That lets the T=512 serial chain be
cut into S=32 time chunks run CONCURRENTLY: each chunk starts from the
uniform state w := e_{t0} a couple of steps (m=2) before its real range and
is correct in *direction* by the time the range starts; its unknown per-batch
log-magnitude offset delta_c is recovered afterwards by matching log-colsums
with the previous chunk at the shared boundary step (a tiny scalar cumsum).

Per core (64-batch shard), the 32 chunks run as 2 pair-groups of 16 batched
into the free axis: two [65, 512] matmuls per pair-step (65th weight column
of ones emits the colsum row Z for free) land in one [65, 1024] PSUM tile,
consumed by a single DVE multiply whose emission operand has a preset ones
row — so Z rides through into the persistent SBUF state ring and is
harvested by ONE gather-DMA per pair after the chain (GPSIMD DMA triggers
cost ~800ns each, so DMA count is minimized everywhere: emissions are
host-prearranged into the exact staging layout and load as two big
contiguous DMAs per pair-window).  Emissions are exp'd on the Scalar engine.
Final combine: per-chunk Z histories are PE-transposed to [b, slot], matched
into delta_c (log-ratio cumsum), and the one-hot time mask (host-preprocessed
into one-hot (chunk,slot) + chunk indicators) selects
ln Z(tau_b) + delta_c(b) + D*tau_b; a ones-matmul reduces the batch on core.

Sharding: batch 512 = 8 cores x 64, transitions/alpha_0 replicated; host sums
the 8 per-core scalars.
"""

import os
import sys

for _p in ("/opt/trn_rl_repo", "/root/.axon_site/_ro/trn_rl_repo"):
    if os.path.isdir(_p) and _p not in sys.path:
        sys.path.insert(0, _p)

from contextlib import ExitStack

import numpy as np

import concourse.bass as bass
import concourse.mybir as mybir
import concourse.tile as tile
from concourse.bass_utils import run_bass_kernel_spmd
from concourse.masks import make_identity

# Walrus in this container rejects instructions with >1 sync-wait; split the
# extras onto preceding same-engine no-ops (queues are in-order, so identical
# semantics).
_ORIG_COMMIT = tile.TileContext._commit_instruction


def _single_wait_commit(self, inst, lazy_reg_writes=True):
    si = getattr(inst, "sync_info", None)
    if (
        si is not None
        and si.on_wait
        and len(si.on_wait) > 1
        and inst.engine != mybir.EngineType.Unassigned
    ):
        waits = list(si.on_wait)
        eng = self.nc.engines[inst.engine]
        for w in waits[:-1]:
            n = eng.nop(nofuse=True)
            n.ins.sync_info = mybir.SyncInfo(on_wait=[w], on_update=[])
        inst.sync_info = mybir.SyncInfo(
            on_wait=[waits[-1]], on_update=list(si.on_update or [])
        )
    _ORIG_COMMIT(self, inst, lazy_reg_writes)


tile.TileContext._commit_instruction = _single_wait_commit

T, B, K = 512, 512, 64
NCORES = 8
BSH = B // NCORES      # 64 batch per core
P = 16                 # real steps per chunk
M = 1                  # burn-in steps
S = T // P             # 32 chunks
LL = P + M             # 17 chain steps per chunk (zbuf rows 1..LL)
NR = LL + 1            # 18 zbuf rows (row 0 unused, kept 1.0)
NR2 = NR              # 18: zT column pitch (even -> 4-byte aligned bf16 PSUM)
NP = 2                 # pair-groups
GP = S // NP           # 16 chunks per pair-group
PC = GP * BSH          # 1024 columns per pair-group
HC = PC // 2           # 512 columns per matmul
W = 3                  # emission window (steps per DMA/exp block)
NW = 6                 # windows cover slots 0..17 (slot 17 is zero padding)
DELTA = 4.0            # per-step log-space offset folded into ETs
F32 = mybir.dt.float32
BF16 = mybir.dt.bfloat16
MULT = mybir.AluOpType.mult
ADD = mybir.AluOpType.add
SUB = mybir.AluOpType.subtract
AX = mybir.AxisListType.X
AF = mybir.ActivationFunctionType


def _t_start(c):
    return 0 if c == 0 else c * P - M


def _build_crf_nc() -> bass.Bass:
    nc = bass.Bass(trn_type="TRN2", target_bir_lowering=False, debug=False)

    # emissions host-prearranged into per-(pair,window) staging blocks:
    # row (p*NW + wv)*K + k, col = step_in_window*PC + chunk_in_pair*BSH + b
    # 65th row is 0.0 so exp() yields the ones row for Z passthrough
    emt_d = nc.dram_tensor(
        "emits_blk", [NP * NW * (K + 1), W * PC], BF16, kind="ExternalInput"
    ).ap()
    trans_d = nc.dram_tensor("transitions", [K, K], F32, kind="ExternalInput").ap()
    alpha0_d = nc.dram_tensor("alpha_0", [K, 1], F32, kind="ExternalInput").ap()
    ohz_d = nc.dram_tensor("onehot_z", [BSH, S * NR2], F32, kind="ExternalInput").ap()
    ohc_d = nc.dram_tensor("onehot_c", [BSH, S], F32, kind="ExternalInput").ap()
    taud_d = nc.dram_tensor("tau_delta", [BSH, 1], F32, kind="ExternalInput").ap()
    out_d = nc.dram_tensor("out_sum", [1, 1], F32, kind="ExternalOutput").ap()

    with tile.TileContext(nc) as tc:
        with ExitStack() as ctx:
            _crf_body(ctx, tc, emt_d, trans_d, alpha0_d, ohz_d, ohc_d, taud_d,
                      out_d)
    _split_remaining_multiwaits(nc)
    return nc


def _split_remaining_multiwaits(nc):
    for blk in nc.m.functions[0].blocks:
        il = blk.instructions
        idx = 0
        while idx < len(il):
            inst = il[idx]
            si = inst.sync_info
            if si is not None and si.on_wait and len(si.on_wait) > 1:
                waits = list(si.on_wait)
                for j, w in enumerate(waits[:-1]):
                    n = mybir.InstNoOp(
                        name=f"I-swx-{inst.name}-{j}", ins=[], outs=[]
                    )
                    n.engine = inst.engine
                    n.sync_info = mybir.SyncInfo(on_wait=[w], on_update=[])
                    nc.register_instruction(n, overwrite=True)
                    il.insert(idx, n)
                    idx += 1
                inst.sync_info = mybir.SyncInfo(
                    on_wait=[waits[-1]], on_update=list(si.on_update or [])
                )
            idx += 1


def _crf_body(ctx, tc, emt_d, trans_d, alpha0_d, ohz_d, ohc_d, taud_d, out_d):
    nc = tc.nc

    # ---- long-lived SBUF ----
    ets = nc.alloc_sbuf_tensor("ets", [K, K + 1], BF16).ap()
    expal = nc.alloc_sbuf_tensor("expal", [K + 1, 1], F32).ap()
    identf = nc.alloc_sbuf_tensor("identf", [NR + 1, NR + 1], BF16).ap()
    ones_b = nc.alloc_sbuf_tensor("ones_b", [BSH, 1], F32).ap()
    cst = nc.alloc_sbuf_tensor("cst", [128, 2], F32).ap()  # col0=0, col1=-DELTA
    zbuf = [
        nc.alloc_sbuf_tensor(f"zbuf{p}", [NR, PC], BF16).ap() for p in range(NP)
    ]
    # state ring: slot s holds w_s [65, PC]; row 64 = Z(s-1) passthrough,
    # harvested by one gather-DMA per pair after the chain.
    wring = [
        nc.alloc_sbuf_tensor(f"wring{p}", [K + 1, (LL + 1) * PC], BF16).ap()
        for p in range(NP)
    ]
    # emission staging: per pair 3 persistent buffers [65, W*PC] bf16 with
    # row 64 = 1.0 (preset once) so the 65-row multiply passes Z through.
    eexp = [
        [nc.alloc_sbuf_tensor(f"eexp{p}_{i}", [K + 1, W * PC], BF16).ap()
         for i in range(3)]
        for p in range(NP)
    ]

    # ---- one-time setup (gpsimd: keeps the DVE queue free at startup) ----
    nc.gpsimd.memset(cst[:, 0:1], 0.0)
    nc.gpsimd.memset(cst[:, 1:2], -DELTA)
    nc.gpsimd.memset(ones_b[:, :], 1.0)
    fin_pool = ctx.enter_context(tc.tile_pool(name="fin", bufs=1))

    fpsum = ctx.enter_context(tc.tile_pool(name="fpsum", bufs=2, space="PSUM"))
    with ExitStack() as chain_ctx:
        raw_pool = chain_ctx.enter_context(tc.tile_pool(name="raw", bufs=3))
        u_psum = chain_ctx.enter_context(
            tc.tile_pool(name="upsum", bufs=1, space="PSUM")
        )

        def load_window(p, wv, nsplit=4, nexp=1):
            rt = raw_pool.tile([K + 1, W * PC], BF16, tag=f"raw{p}")
            r0 = (p * NW + wv) * (K + 1)
            q = W * PC // nsplit
            for i in range(nsplit):
                eng = nc.gpsimd if i % 2 == 0 else nc.sync
                eng.dma_start(
                    rt[:, i * q : (i + 1) * q],
                    emt_d[r0 : r0 + K + 1, i * q : (i + 1) * q],
                )
            dst = eexp[p][wv % 3]
            e = W * PC // nexp
            for i in range(nexp):
                nc.scalar.activation(
                    dst[:, i * e : (i + 1) * e],
                    rt[:, i * e : (i + 1) * e],
                    AF.Exp,
                    bias=cst[0 : K + 1, 0:1],
                )

        for p in range(NP):
            load_window(p, 0, nsplit=8, nexp=2)
        # transitions/alpha go right after window 0's triggers
        tr_t = fin_pool.tile([K, K], F32, tag="trt")
        nc.sync.dma_start(tr_t[:], trans_d)
        nc.scalar.activation(ets[:, 0:K], tr_t[:], AF.Exp, bias=cst[0:K, 1:2])
        nc.vector.memset(ets[:, K : K + 1], 1.0)
        a0_t = fin_pool.tile([K, 1], F32, tag="a0t")
        nc.gpsimd.dma_start(a0_t[:], alpha0_d)
        nc.scalar.activation(expal[0:K], a0_t[:], AF.Exp, bias=cst[0:K, 0:1])
        nc.vector.memset(expal[K : K + 1], 1.0)
        for wv in range(1, 3):
            for p in range(NP):
                load_window(p, wv)
        # bulky one-time setup AFTER the loads so it never delays them
        for p in range(NP):
            nc.gpsimd.memset(zbuf[p][0:1, :], 1.0)  # row 0 -> ln = 0
        make_identity(nc, identf)
        ohz = fin_pool.tile([BSH, S * NR2], F32, tag="ohz")
        nc.sync.dma_start(ohz[:], ohz_d)
        ohc = fin_pool.tile([BSH, S], F32, tag="ohc")
        nc.sync.dma_start(ohc[:], ohc_d)
        taud = fin_pool.tile([BSH, 1], F32, tag="taud")
        nc.sync.dma_start(taud[:], taud_d)

        # init states: w0 = e_{t0} (chunks >=1), chunk 0: expal * e_0
        for p in range(NP):
            wt = wring[p][:, 0:PC]
            sv = eexp[p][0][:, 0:PC]
            if p == 0:
                nc.vector.tensor_scalar(
                    wt[:, 0:BSH], sv[:, 0:BSH], expal, None, op0=MULT
                )
                nc.vector.tensor_copy(wt[:, BSH:PC], sv[:, BSH:PC])
            else:
                nc.vector.tensor_copy(wt[:, :], sv[:, :])

        # ---- chain: steps 1..LL ----
        for s in range(1, LL + 1):
            if s % W == 0 and s // W + 3 <= NW:
                for p in range(NP):
                    load_window(p, s // W + 2)
            se = min(s, LL - 1)        # step LL reuses step LL-1's emission
            wv, sw = se // W, se % W
            for p in range(NP):
                u = u_psum.tile([K + 1, PC], F32, tag=f"u{p}")
                nc.tensor.matmul(
                    u[:, 0:HC],
                    ets[:, :],
                    wring[p][0:K, (s - 1) * PC : (s - 1) * PC + HC],
                    start=True,
                    stop=True,
                )
                nc.tensor.matmul(
                    u[:, HC:PC],
                    ets[:, :],
                    wring[p][0:K, (s - 1) * PC + HC : s * PC],
                    start=True,
                    stop=True,
                )
                nc.vector.tensor_tensor(
                    wring[p][:, s * PC : (s + 1) * PC],
                    u[:, :],
                    eexp[p][wv % 3][:, sw * PC : (sw + 1) * PC],
                    op=MULT,
                )
        # Z harvest: row 64 of slots 1..LL -> zbuf rows 1..LL (one DMA/pair)
        for p in range(NP):
            nc.gpsimd.dma_start(
                zbuf[p][1 : LL + 1, :],
                wring[p][K : K + 1, PC : (LL + 1) * PC].rearrange(
                    "r (s c) -> r s c", s=LL
                ),
            )

    # ---- final combine ----
    zT = fin_pool.tile([BSH, S * NR2], F32, tag="zT")
    # pad columns would otherwise hold junk; preset whole tile Ln-safe
    nc.vector.memset(zT[:, :], 1.0)
    for h in range(2):
        zt = fpsum.tile([BSH, (S // 2) * NR2], BF16, tag="zt")
        for ci in range(S // 2):
            c = h * (S // 2) + ci
            p, gi = c // GP, c % GP
            nc.tensor.transpose(
                zt[:, ci * NR2 : ci * NR2 + NR],
                zbuf[p][:, gi * BSH : (gi + 1) * BSH],
                identf[0:NR, 0:NR],
            )
        nc.vector.tensor_copy(
            zT[:, h * (S // 2) * NR2 : (h + 1) * (S // 2) * NR2].rearrange(
                "b (c r) -> b c r", r=NR2
            )[:, :, 0:NR],
            zt[:].rearrange("b (c r) -> b c r", r=NR2)[:, :, 0:NR],
        )
    # patch: chunk0's matching column (row LL) := its row P (t = P-1)
    nc.vector.tensor_copy(zT[:, LL : LL + 1], zT[:, P : P + 1])
    lnz = fin_pool.tile([BSH, S * NR2], F32, tag="lnz")
    nc.scalar.activation(lnz[:], zT[:], AF.Ln, bias=cst[0:BSH, 0:1])

    # delta stitching: inc[:, i] = lnz[:, NR2*(i-1) + LL] - lnz[:, NR2*i + M]
    lv = lnz[:].rearrange("b (c r) -> b c r", r=NR2)
    inc = fin_pool.tile([BSH, S], F32, tag="inc")
    nc.vector.memset(inc[:, 0:1], 0.0)
    nc.vector.tensor_tensor(
        inc[:, 1:S], lv[:, 0 : S - 1, LL], lv[:, 1:S, M], op=SUB
    )
    # cumulative sum over chunks (Hillis-Steele, ping-pong)
    cs_a = inc
    for k in (1, 2, 4, 8, 16):
        cs_b = fin_pool.tile([BSH, S], F32, tag=f"cs{k}")
        nc.vector.tensor_copy(cs_b[:, 0:k], cs_a[:, 0:k])
        nc.vector.tensor_tensor(
            cs_b[:, k:S], cs_a[:, k:S], cs_a[:, 0 : S - k], op=ADD
        )
        cs_a = cs_b

    scr1 = fin_pool.tile([BSH, S * NR2], F32, tag="scr1")
    zsel = fin_pool.tile([BSH, 1], F32, tag="zsel")
    nc.vector.tensor_tensor(scr1[:], lnz[:], ohz[:], op=MULT)
    nc.vector.tensor_reduce(zsel[:], scr1[:], axis=AX, op=ADD)
    scr2 = fin_pool.tile([BSH, S], F32, tag="scr2")
    dsel = fin_pool.tile([BSH, 1], F32, tag="dsel")
    nc.vector.tensor_tensor(scr2[:], cs_a[:], ohc[:], op=MULT)
    nc.vector.tensor_reduce(dsel[:], scr2[:], axis=AX, op=ADD)
    res = fin_pool.tile([BSH, 1], F32, tag="res")
    nc.vector.tensor_tensor(res[:], zsel[:], dsel[:], op=ADD)
    nc.vector.tensor_tensor(res[:], res[:], taud[:], op=ADD)
    acc = fpsum.tile([1, 1], F32, tag="acc", bufs=1)
    nc.tensor.matmul(acc[:], res[:], ones_b[:], start=True, stop=True)
    osb = fin_pool.tile([1, 1], F32, tag="osb")
    nc.scalar.copy(osb[:], acc[:])
    nc.sync.dma_start(out_d, osb[:])


_NC_CACHE = None


def _get_nc():
    global _NC_CACHE
    if _NC_CACHE is None:
        _NC_CACHE = _build_crf_nc()
    return _NC_CACHE


def _make_in_maps(np_inputs):
    import ml_dtypes

    emits = np.asarray(np_inputs["emits"], dtype=np.float32)
    mask = np.asarray(np_inputs["mask"])
    transitions = np.asarray(np_inputs["transitions"], dtype=np.float32)
    alpha_0 = np.asarray(np_inputs["alpha_0"], dtype=np.float32)
    emits_t = emits.transpose(0, 2, 1)  # [T, K, B] view
    tau = mask.argmax(0).astype(np.int64)  # [B]
    chunk = tau // P
    row = np.where(chunk == 0, tau + 1, tau % P + M + 1)
    in_maps = []
    for cix in range(NCORES):
        sl = slice(cix * BSH, (cix + 1) * BSH)
        tau_s, c_s, r_s = tau[sl], chunk[sl], row[sl]
        ohz = np.zeros((BSH, S * NR2), dtype=np.float32)
        ohz[np.arange(BSH), c_s * NR2 + r_s] = 1.0
        ohc = np.zeros((BSH, S), dtype=np.float32)
        ohc[np.arange(BSH), c_s] = 1.0
        taud = (DELTA * tau_s).astype(np.float32).reshape(BSH, 1)
        sh = emits_t[:, :, sl]  # [T, K, 64]
        # staging blocks [pair, window, k(+zero row), step, chunk_in_pair, b]
        nslot = NW * W
        blk = np.zeros((NP, NW, K + 1, W, GP, BSH), dtype=np.float32)
        for p in range(NP):
            for ci in range(GP):
                t0 = _t_start(p * GP + ci)
                ns = min(nslot, T - t0)
                sv = np.zeros((nslot, K, BSH), dtype=np.float32)
                sv[:ns] = sh[t0 : t0 + ns]
                blk[p, :, 0:K, :, ci, :] = (
                    sv.reshape(NW, W, K, BSH).transpose(0, 2, 1, 3)
                )
        emb = blk.reshape(NP * NW * (K + 1), W * PC).astype(ml_dtypes.bfloat16)
        in_maps.append(
            {
                "emits_blk": emb,
                "transitions": transitions,
                "alpha_0": alpha_0,
                "onehot_z": ohz,
                "onehot_c": ohc,
                "tau_delta": taud,
            }
        )
    return in_maps


def kernel(emits, mask, transitions, alpha_0):
    nc = _get_nc()
    in_maps = _make_in_maps(
        {"emits": emits, "mask": mask, "transitions": transitions,
         "alpha_0": alpha_0}
    )
    res = run_bass_kernel_spmd(nc, in_maps, core_ids=list(range(NCORES)))
    total = np.float64(0.0)
    for r in res.results:
        total += np.asarray(r["out_sum"], dtype=np.float64).sum()
    return np.float32(total)


# revision 21
# speedup vs baseline: 4.6065x; 1.0255x over previous
"""CRF forward (logsumexp over paths) loss kernel for Trainium2, 8 NeuronCores.

Time-parallel chunked algorithm
-------------------------------
The linear-space recurrence  w_t = (ETs^T w_{t-1}) * e_t  (ETs = exp(trans-D),
e_t = exp(emit_t), state [K, B] per core) is a product of positive matrices,
so it forgets its initial condition at the Birkhoff contraction rate —
measured here at ~2 decades per 2 steps.  That lets the T=512 serial chain be
cut into S=32 time chunks run CONCURRENTLY: each chunk starts from the
uniform state w := e_{t0} a couple of steps (m=2) before its real range and
is correct in *direction* by the time the range starts; its unknown per-batch
log-magnitude offset delta_c is recovered afterwards by matching log-colsums
with the previous chunk at the shared boundary step (a tiny scalar cumsum).

Per core (64-batch shard), the 32 chunks run as 2 pair-groups of 16 batched
into the free axis: two [65, 512] matmuls per pair-step (65th weight column
of ones emits the colsum row Z for free) land in one [65, 1024] PSUM tile,
consumed by a single DVE multiply whose emission operand has a preset ones
row — so Z rides through into the persistent SBUF state ring and is
harvested by ONE gather-DMA per pair after the chain (GPSIMD DMA triggers
cost ~800ns each, so DMA count is minimized everywhere: emissions are
host-prearranged into the exact staging layout and load as two big
contiguous DMAs per pair-window).  Emissions are exp'd on the Scalar engine.
Final combine: per-chunk Z histories are PE-transposed to [b, slot], matched
into delta_c (log-ratio cumsum), and the one-hot time mask (host-preprocessed
into one-hot (chunk,slot) + chunk indicators) selects
ln Z(tau_b) + delta_c(b) + D*tau_b; a ones-matmul reduces the batch on core.

Sharding: batch 512 = 8 cores x 64, transitions/alpha_0 replicated; host sums
the 8 per-core scalars.
"""

import os
import sys

for _p in ("/opt/trn_rl_repo", "/root/.axon_site/_ro/trn_rl_repo"):
    if os.path.isdir(_p) and _p not in sys.path:
        sys.path.insert(0, _p)

from contextlib import ExitStack

import numpy as np

import concourse.bass as bass
import concourse.mybir as mybir
import concourse.tile as tile
from concourse.bass_utils import run_bass_kernel_spmd
from concourse.masks import make_identity

# Walrus in this container rejects instructions with >1 sync-wait; split the
# extras onto preceding same-engine no-ops (queues are in-order, so identical
# semantics).
_ORIG_COMMIT = tile.TileContext._commit_instruction


def _single_wait_commit(self, inst, lazy_reg_writes=True):
    si = getattr(inst, "sync_info", None)
    if (
        si is not None
        and si.on_wait
        and len(si.on_wait) > 1
        and inst.engine != mybir.EngineType.Unassigned
    ):
        waits = list(si.on_wait)
        eng = self.nc.engines[inst.engine]
        for w in waits[:-1]:
            n = eng.nop(nofuse=True)
            n.ins.sync_info = mybir.SyncInfo(on_wait=[w], on_update=[])
        inst.sync_info = mybir.SyncInfo(
            on_wait=[waits[-1]], on_update=list(si.on_update or [])
        )
    _ORIG_COMMIT(self, inst, lazy_reg_writes)


tile.TileContext._commit_instruction = _single_wait_commit

T, B, K = 512, 512, 64
NCORES = 8
BSH = B // NCORES      # 64 batch per core
P = 16                 # real steps per chunk
M = 1                  # burn-in steps
S = T // P             # 32 chunks
LL = P + M             # 17 chain steps per chunk (zbuf rows 1..LL)
NR = LL + 1            # 18 zbuf rows (row 0 unused, kept 1.0)
NR2 = NR              # 18: zT column pitch (even -> 4-byte aligned bf16 PSUM)
NP = 2                 # pair-groups
GP = S // NP           # 16 chunks per pair-group
PC = GP * BSH          # 1024 columns per pair-group
HC = PC // 2           # 512 columns per matmul
W = 3                  # emission window (steps per DMA/exp block)
NW = 6                 # windows cover slots 0..17 (slot 17 is zero padding)
DELTA = 4.0            # per-step log-space offset folded into ETs
F32 = mybir.dt.float32
BF16 = mybir.dt.bfloat16
MULT = mybir.AluOpType.mult
ADD = mybir.AluOpType.add
SUB = mybir.AluOpType.subtract
AX = mybir.AxisListType.X
AF = mybir.ActivationFunctionType


def _t_start(c):
    return 0 if c == 0 else c * P - M


def _build_crf_nc() -> bass.Bass:
    nc = bass.Bass(trn_type="TRN2", target_bir_lowering=False, debug=False)

    # emissions host-prearranged into per-(pair,window) staging blocks:
    # row (p*NW + wv)*K + k, col = step_in_window*PC + chunk_in_pair*BSH + b
    # 65th row is 0.0 so exp() yields the ones row for Z passthrough
    emt_d = nc.dram_tensor(
        "emits_blk", [NP * NW * (K + 1), W * PC], BF16, kind="ExternalInput"
    ).ap()
    trans_d = nc.dram_tensor("transitions", [K, K], F32, kind="ExternalInput").ap()
    alpha0_d = nc.dram_tensor("alpha_0", [K, 1], F32, kind="ExternalInput").ap()
    ohz_d = nc.dram_tensor("onehot_z", [BSH, S * NR2], F32, kind="ExternalInput").ap()
    ohc_d = nc.dram_tensor("onehot_c", [BSH, S], F32, kind="ExternalInput").ap()
    taud_d = nc.dram_tensor("tau_delta", [BSH, 1], F32, kind="ExternalInput").ap()
    out_d = nc.dram_tensor("out_sum", [1, 1], F32, kind="ExternalOutput").ap()

    with tile.TileContext(nc) as tc:
        with ExitStack() as ctx:
            _crf_body(ctx, tc, emt_d, trans_d, alpha0_d, ohz_d, ohc_d, taud_d,
                      out_d)
    _split_remaining_multiwaits(nc)
    return nc


def _split_remaining_multiwaits(nc):
    for blk in nc.m.functions[0].blocks:
        il = blk.instructions
        idx = 0
        while idx < len(il):
            inst = il[idx]
            si = inst.sync_info
            if si is not None and si.on_wait and len(si.on_wait) > 1:
                waits = list(si.on_wait)
                for j, w in enumerate(waits[:-1]):
                    n = mybir.InstNoOp(
                        name=f"I-swx-{inst.name}-{j}", ins=[], outs=[]
                    )
                    n.engine = inst.engine
                    n.sync_info = mybir.SyncInfo(on_wait=[w], on_update=[])
                    nc.register_instruction(n, overwrite=True)
                    il.insert(idx, n)
                    idx += 1
                inst.sync_info = mybir.SyncInfo(
                    on_wait=[waits[-1]], on_update=list(si.on_update or [])
                )
            idx += 1


def _crf_body(ctx, tc, emt_d, trans_d, alpha0_d, ohz_d, ohc_d, taud_d, out_d):
    nc = tc.nc

    # ---- long-lived SBUF ----
    ets = nc.alloc_sbuf_tensor("ets", [K, K + 1], BF16).ap()
    expal = nc.alloc_sbuf_tensor("expal", [K + 1, 1], F32).ap()
    identf = nc.alloc_sbuf_tensor("identf", [NR + 1, NR + 1], BF16).ap()
    ones_b = nc.alloc_sbuf_tensor("ones_b", [BSH, 1], F32).ap()
    cst = nc.alloc_sbuf_tensor("cst", [128, 2], F32).ap()  # col0=0, col1=-DELTA
    zbuf = [
        nc.alloc_sbuf_tensor(f"zbuf{p}", [NR, PC], BF16).ap() for p in range(NP)
    ]
    # state ring: slot s holds w_s [65, PC]; row 64 = Z(s-1) passthrough,
    # harvested by one gather-DMA per pair after the chain.
    wring = [
        nc.alloc_sbuf_tensor(f"wring{p}", [K + 1, (LL + 1) * PC], BF16).ap()
        for p in range(NP)
    ]
    # emission staging: per pair 3 persistent buffers [65, W*PC] bf16 with
    # row 64 = 1.0 (preset once) so the 65-row multiply passes Z through.
    eexp = [
        [nc.alloc_sbuf_tensor(f"eexp{p}_{i}", [K + 1, W * PC], BF16).ap()
         for i in range(3)]
        for p in range(NP)
    ]

    # ---- one-time setup (gpsimd: keeps the DVE queue free at startup) ----
    nc.gpsimd.memset(cst[:, 0:1], 0.0)
    nc.gpsimd.memset(cst[:, 1:2], -DELTA)
    nc.gpsimd.memset(ones_b[:, :], 1.0)
    fin_pool = ctx.enter_context(tc.tile_pool(name="fin", bufs=1))

    fpsum = ctx.enter_context(tc.tile_pool(name="fpsum", bufs=2, space="PSUM"))
    with ExitStack() as chain_ctx:
        raw_pool = chain_ctx.enter_context(tc.tile_pool(name="raw", bufs=3))
        u_psum = chain_ctx.enter_context(
            tc.tile_pool(name="upsum", bufs=1, space="PSUM")
        )

        def load_window(p, wv, nsplit=4, nexp=1):
            rt = raw_pool.tile([K + 1, W * PC], BF16, tag=f"raw{p}")
            r0 = (p * NW + wv) * (K + 1)
            q = W * PC // nsplit
            for i in range(nsplit):
                eng = nc.gpsimd if i % 2 == 0 else nc.sync
                eng.dma_start(
                    rt[:, i * q : (i + 1) * q],
                    emt_d[r0 : r0 + K + 1, i * q : (i + 1) * q],
                )
            dst = eexp[p][wv % 3]
            e = W * PC // nexp
            for i in range(nexp):
                nc.scalar.activation(
                    dst[:, i * e : (i + 1) * e],
                    rt[:, i * e : (i + 1) * e],
                    AF.Exp,
                    bias=cst[0 : K + 1, 0:1],
                )

        a0_t = fin_pool.tile([K, 1], F32, tag="a0t")
        nc.gpsimd.dma_start(a0_t[:], alpha0_d)
        for p in range(NP):
            load_window(p, 0, nsplit=8, nexp=3)
        # transitions/alpha go right after window 0's triggers
        tr_t = fin_pool.tile([K, K], F32, tag="trt")
        nc.sync.dma_start(tr_t[:], trans_d)
        nc.scalar.activation(ets[:, 0:K], tr_t[:], AF.Exp, bias=cst[0:K, 1:2])
        nc.vector.memset(ets[:, K : K + 1], 1.0)
        nc.scalar.activation(expal[0:K], a0_t[:], AF.Exp, bias=cst[0:K, 0:1])
        nc.vector.memset(expal[K : K + 1], 1.0)
        for wv in range(1, 3):
            for p in range(NP):
                load_window(p, wv)
        # bulky one-time setup AFTER the loads so it never delays them
        for p in range(NP):
            nc.gpsimd.memset(zbuf[p][0:1, :], 1.0)  # row 0 -> ln = 0
        make_identity(nc, identf)
        ohz = fin_pool.tile([BSH, S * NR2], F32, tag="ohz")
        nc.sync.dma_start(ohz[:], ohz_d)
        ohc = fin_pool.tile([BSH, S], F32, tag="ohc")
        nc.sync.dma_start(ohc[:], ohc_d)
        taud = fin_pool.tile([BSH, 1], F32, tag="taud")
        nc.sync.dma_start(taud[:], taud_d)

        # init states: w0 = e_{t0} (chunks >=1), chunk 0: expal * e_0
        for p in range(NP):
            wt = wring[p][:, 0:PC]
            sv = eexp[p][0][:, 0:PC]
            if p == 0:
                nc.vector.tensor_scalar(
                    wt[:, 0:BSH], sv[:, 0:BSH], expal, None, op0=MULT
                )
                nc.vector.tensor_copy(wt[:, BSH:PC], sv[:, BSH:PC])
            else:
                nc.vector.tensor_copy(wt[:, :], sv[:, :])

        # ---- chain: steps 1..LL ----
        for s in range(1, LL + 1):
            if s % W == 0 and s // W + 3 <= NW:
                for p in range(NP):
                    load_window(p, s // W + 2)
            se = min(s, LL - 1)        # step LL reuses step LL-1's emission
            wv, sw = se // W, se % W
            for p in range(NP):
                u = u_psum.tile([K + 1, PC], F32, tag=f"u{p}")
                nc.tensor.matmul(
                    u[:, 0:HC],
                    ets[:, :],
                    wring[p][0:K, (s - 1) * PC : (s - 1) * PC + HC],
                    start=True,
                    stop=True,
                )
                nc.tensor.matmul(
                    u[:, HC:PC],
                    ets[:, :],
                    wring[p][0:K, (s - 1) * PC + HC : s * PC],
                    start=True,
                    stop=True,
                )
                nc.vector.tensor_tensor(
                    wring[p][:, s * PC : (s + 1) * PC],
                    u[:, :],
                    eexp[p][wv % 3][:, sw * PC : (sw + 1) * PC],
                    op=MULT,
                )
                if s == LL:
                    # Z harvest: row 64 of slots 1..LL -> zbuf rows 1..LL
                    nc.gpsimd.dma_start(
                        zbuf[p][1 : LL + 1, :],
                        wring[p][K : K + 1, PC : (LL + 1) * PC].rearrange(
                            "r (s c) -> r s c", s=LL
                        ),
                    )


    # ---- final combine ----
    zT = fin_pool.tile([BSH, S * NR2], F32, tag="zT")
    # pad columns would otherwise hold junk; preset whole tile Ln-safe
    nc.vector.memset(zT[:, :], 1.0)
    for h in range(2):
        zt = fpsum.tile([BSH, (S // 2) * NR2], BF16, tag="zt")
        for ci in range(S // 2):
            c = h * (S // 2) + ci
            p, gi = c // GP, c % GP
            nc.tensor.transpose(
                zt[:, ci * NR2 : ci * NR2 + NR],
                zbuf[p][:, gi * BSH : (gi + 1) * BSH],
                identf[0:NR, 0:NR],
            )
        nc.vector.tensor_copy(
            zT[:, h * (S // 2) * NR2 : (h + 1) * (S // 2) * NR2].rearrange(
                "b (c r) -> b c r", r=NR2
            )[:, :, 0:NR],
            zt[:].rearrange("b (c r) -> b c r", r=NR2)[:, :, 0:NR],
        )
    # patch: chunk0's matching column (row LL) := its row P (t = P-1)
    nc.vector.tensor_copy(zT[:, LL : LL + 1], zT[:, P : P + 1])
    lnz = fin_pool.tile([BSH, S * NR2], F32, tag="lnz")
    nc.scalar.activation(lnz[:], zT[:], AF.Ln, bias=cst[0:BSH, 0:1])

    # delta stitching: inc[:, i] = lnz[:, NR2*(i-1) + LL] - lnz[:, NR2*i + M]
    lv = lnz[:].rearrange("b (c r) -> b c r", r=NR2)
    inc = fin_pool.tile([BSH, S], F32, tag="inc")
    nc.vector.memset(inc[:, 0:1], 0.0)
    nc.vector.tensor_tensor(
        inc[:, 1:S], lv[:, 0 : S - 1, LL], lv[:, 1:S, M], op=SUB
    )
    scr1 = fin_pool.tile([BSH, S * NR2], F32, tag="scr1")
    zsel = fin_pool.tile([BSH, 1], F32, tag="zsel")
    nc.vector.tensor_tensor(scr1[:], lnz[:], ohz[:], op=MULT)
    nc.vector.tensor_reduce(zsel[:], scr1[:], axis=AX, op=ADD)
    # ohc is a step mask (1 for c <= chunk(tau_b)), so the cumulative-sum
    # of boundary increments folds into this single select-reduce.
    scr2 = fin_pool.tile([BSH, S], F32, tag="scr2")
    dsel = fin_pool.tile([BSH, 1], F32, tag="dsel")
    nc.vector.tensor_tensor(scr2[:], inc[:], ohc[:], op=MULT)
    nc.vector.tensor_reduce(dsel[:], scr2[:], axis=AX, op=ADD)
    res = fin_pool.tile([BSH, 1], F32, tag="res")
    nc.vector.tensor_tensor(res[:], zsel[:], dsel[:], op=ADD)
    nc.vector.tensor_tensor(res[:], res[:], taud[:], op=ADD)
    acc = fpsum.tile([1, 1], F32, tag="acc", bufs=1)
    nc.tensor.matmul(acc[:], res[:], ones_b[:], start=True, stop=True)
    osb = fin_pool.tile([1, 1], F32, tag="osb")
    nc.scalar.copy(osb[:], acc[:])
    nc.sync.dma_start(out_d, osb[:])


_NC_CACHE = None


def _get_nc():
    global _NC_CACHE
    if _NC_CACHE is None:
        _NC_CACHE = _build_crf_nc()
    return _NC_CACHE


def _make_in_maps(np_inputs):
    import ml_dtypes

    emits = np.asarray(np_inputs["emits"], dtype=np.float32)
    mask = np.asarray(np_inputs["mask"])
    transitions = np.asarray(np_inputs["transitions"], dtype=np.float32)
    alpha_0 = np.asarray(np_inputs["alpha_0"], dtype=np.float32)
    emits_t = emits.transpose(0, 2, 1)  # [T, K, B] view
    tau = mask.argmax(0).astype(np.int64)  # [B]
    chunk = tau // P
    row = np.where(chunk == 0, tau + 1, tau % P + M + 1)
    in_maps = []
    for cix in range(NCORES):
        sl = slice(cix * BSH, (cix + 1) * BSH)
        tau_s, c_s, r_s = tau[sl], chunk[sl], row[sl]
        ohz = np.zeros((BSH, S * NR2), dtype=np.float32)
        ohz[np.arange(BSH), c_s * NR2 + r_s] = 1.0
        ohc = (np.arange(S)[None, :] <= c_s[:, None]).astype(np.float32)
        taud = (DELTA * tau_s).astype(np.float32).reshape(BSH, 1)
        sh = emits_t[:, :, sl]  # [T, K, 64]
        # staging blocks [pair, window, k(+zero row), step, chunk_in_pair, b]
        nslot = NW * W
        blk = np.zeros((NP, NW, K + 1, W, GP, BSH), dtype=np.float32)
        for p in range(NP):
            for ci in range(GP):
                t0 = _t_start(p * GP + ci)
                ns = min(nslot, T - t0)
                sv = np.zeros((nslot, K, BSH), dtype=np.float32)
                sv[:ns] = sh[t0 : t0 + ns]
                blk[p, :, 0:K, :, ci, :] = (
                    sv.reshape(NW, W, K, BSH).transpose(0, 2, 1, 3)
                )
        emb = blk.reshape(NP * NW * (K + 1), W * PC).astype(ml_dtypes.bfloat16)
        in_maps.append(
            {
                "emits_blk": emb,
                "transitions": transitions,
                "alpha_0": alpha_0,
                "onehot_z": ohz,
                "onehot_c": ohc,
                "tau_delta": taud,
            }
        )
    return in_maps


def kernel(emits, mask, transitions, alpha_0):
    nc = _get_nc()
    in_maps = _make_in_maps(
        {"emits": emits, "mask": mask, "transitions": transitions,
         "alpha_0": alpha_0}
    )
    res = run_bass_kernel_spmd(nc, in_maps, core_ids=list(range(NCORES)))
    total = np.float64(0.0)
    for r in res.results:
        total += np.asarray(r["out_sum"], dtype=np.float64).sum()
    return np.float32(total)
